# revision 10
# baseline (speedup 1.0000x reference)
"""Two-layer GATv2 (heads=1, edge_dim=1) on 8 Trainium2 NeuronCores.

Sharding: nodes dealt round-robin by in-degree onto 8 cores; dst-grouped
edges stay local; source features come from an AllGather'd full table via
dma_gather on three concurrent SWDGE queues (queues 1-3 dispatch in ~0.4us
and generate descriptors on their own Q7 core pairs in parallel).

Per 128-node (dst) block:
  v   = [xrT | w]-lhsT @ [IdTile | WeDiag] + ident @ g    -- 2 matmuls/chunk
  m   = prelu(v, 0.2)                                     -- ACT (from PSUM)
  s   = sum_{d<P1} m - sum_{d>=P1} m                      -- att sign-split
  e   = exp(s), Z = row-sum(e)                            -- ACT with accum
  agg = sum_k e_k * g_k                                   -- DVE mult+reduce
  out = agg * inva * (1/Z) + bias                         -- undo |att| scale
Padded gather slots point at a per-core "fake" table row holding -/+1e6 in
the sign-sorted layout, so exp(s)==0 exactly -- no mask tensors needed.
Dense phase computes xl node-major (lhsT = xT block) and xr feature-major
(lhsT = WrT) directly -- no PE transposes; biases fold into rank-1 matmul /
ACT bias. Layer-2 softplus runs on ACT; the +1e-4 is added host-side.
"""

import numpy as np

N, E, D_IN, DH, DO = 50000, 800000, 128, 64, 32
C = 8                      # cores
NL = N // C                # nodes per core (6250)
P = 128                    # partitions = nodes per block
NB = (NL + P - 1) // P     # blocks per core (49)
NLP = NB * P               # padded nodes per core (6272)
NLF = 6256                 # bounce rows per core (6250 real + fake + pad)
SPLIT = 32768              # int16 gather table split (table-row space)
NT = C * NLF               # table rows (50048)
VSMALL = 32                # K threshold for double-buffered PSUM v-tiles
FAKE_A = 6250              # core-0 fake row (side A)
FAKE_B = 5 * NLF + 6250 - SPLIT   # core-5 fake row offset in side B
GQ = (1, 2, 3, 0)          # SWDGE queues (0 last: it blocks the sequencer)


# ----------------------------------------------------------------------------
# host-side: weight folding and graph layout
# ----------------------------------------------------------------------------

def _fold(Wl, bl, Wr, br, We, att, bias, in_perm=None, h_offset=False):
    att = np.asarray(att, np.float64)
    pi = np.concatenate([np.nonzero(att >= 0)[0], np.nonzero(att < 0)[0]])
    p1 = int((att >= 0).sum())
    a = np.maximum(np.abs(att[pi]), 1e-30)
    Wl = np.asarray(Wl, np.float64)[pi] * a[:, None]
    Wr = np.asarray(Wr, np.float64)[pi] * a[:, None]
    bl = np.asarray(bl, np.float64)[pi] * a
    br = np.asarray(br, np.float64)[pi] * a
    We_ = np.asarray(We, np.float64)[pi, 0] * a
    if in_perm is not None:
        Wl = Wl[:, in_perm]
        Wr = Wr[:, in_perm]
    if h_offset:  # input arrives as h+1
        bl = bl - Wl.sum(1)
        br = br - Wr.sum(1)
    return dict(
        WlT=np.ascontiguousarray(Wl.T, np.float32),
        WrT=np.ascontiguousarray(Wr.T, np.float32),
        blRow=bl.astype(np.float32)[None, :],
        br=br.astype(np.float32)[:, None],
        brRow=br.astype(np.float32)[None, :],
        We=We_.astype(np.float32),
        inva=(1.0 / a).astype(np.float32),
        bias=np.asarray(bias, np.float64)[pi].astype(np.float32),
        pi=pi, p1=p1,
    )


def _prep(x, edge_index, edge_weight):
    src = np.asarray(edge_index[0], np.int64)
    dst = np.asarray(edge_index[1], np.int64)
    w = np.asarray(edge_weight, np.float32)

    deg = np.bincount(dst, minlength=N)
    wsum = np.bincount(dst, weights=w.astype(np.float64), minlength=N)
    loop_w = (wsum / np.maximum(deg, 1)).astype(np.float32)

    order = np.argsort(-deg, kind="stable")
    ranks = np.empty(N, np.int64)
    ranks[order] = np.arange(N)
    core = ranks % C
    crank = ranks // C
    new_id = core * NL + crank          # output-row space
    trow = core * NLF + crank           # gather-table-row space
    inv = np.empty(N, np.int64)
    inv[new_id] = np.arange(N)          # old id of each new id

    esrc = np.concatenate([trow[src], trow])          # table rows of sources
    edst = np.concatenate([new_id[dst], new_id])      # local ids of dsts
    ew = np.concatenate([w, loop_w]).astype(np.float32)

    side = (esrc >= SPLIT).astype(np.int64)
    eord = np.argsort(edst * 2 + side, kind="stable")
    sdst, ssrc, sw, sside = edst[eord], esrc[eord], ew[eord], side[eord]

    nA = np.bincount(edst[side == 0], minlength=N)
    nB = np.bincount(edst[side == 1], minlength=N)

    grp = np.searchsorted(sdst, np.arange(N))
    pos = np.arange(E + N) - grp[sdst]
    posAB = np.where(sside == 0, pos, pos - nA[sdst])

    nblk = (np.arange(N) % NL) // P      # block of each new id
    KA = np.zeros(NB, np.int64)
    KB = np.zeros(NB, np.int64)
    np.maximum.at(KA, nblk, nA)
    np.maximum.at(KB, nblk, nB)
    KA = np.maximum(KA, 1)
    K = KA + KB
    assert int(K.max()) * DH * 4 <= 16384, f"KMAX {K.max()} overflows PSUM"
    KMAX = int(K.max())
    totK = int(K.sum())

    e_core = sdst // NL
    e_loc = sdst % NL
    e_blk = e_loc // P
    e_p = e_loc % P
    e_k = np.where(sside == 0, posAB, KA[e_blk] + posAB)

    colsA = np.concatenate([[0], np.cumsum(KA * 8)]).astype(np.int64)
    colsB = np.concatenate([[0], np.cumsum(KB * 8)]).astype(np.int64)
    idxA = np.full((C, 128, int(colsA[-1])), FAKE_A, np.int16)
    idxB = np.full((C, 128, max(int(colsB[-1]), 16)), FAKE_B, np.int16)
    mA = sside == 0
    fA = e_k[mA] * P + e_p[mA]
    idxA[e_core[mA], fA % 16, colsA[e_blk[mA]] + fA // 16] = \
        ssrc[mA].astype(np.int16)
    mB = ~mA
    fB = (e_k[mB] - KA[e_blk[mB]]) * P + e_p[mB]
    idxB[e_core[mB], fB % 16, colsB[e_blk[mB]] + fB // 16] = \
        (ssrc[mB] - SPLIT).astype(np.int16)
    if NL % P:  # dummy partitions in last block: avoid Z=0 (point at row 0)
        for p in range(NL % P, P):
            f = 0 * P + p
            idxA[:, f % 16, colsA[NB - 1] + f // 16] = 0
    for rep in range(1, 8):
        idxA[:, 16 * rep:16 * rep + 16] = idxA[:, :16]
        idxB[:, 16 * rep:16 * rep + 16] = idxB[:, :16]

    x = np.asarray(x, np.float32)
    xT = np.zeros((C, D_IN, NLP), np.float32)
    perm = inv.reshape(C, NL)
    for c in range(C):
        xT[c, :, :NL] = x[perm[c]].T

    wT = np.zeros((C, KMAX, NLP), np.float16)
    wT[e_core, e_k, e_blk * P + e_p] = sw.astype(np.float16)

    return dict(new_id=new_id, K=K, KA=KA, KB=KB, KMAX=KMAX, totK=totK,
                colsA=colsA, colsB=colsB, idxA=idxA, idxB=idxB, xT=xT, wT=wT)


def _consts(KMAX, We1, We2, p1, p2):
    # layer 1: rhsCat1 = [IdTile1 (64 rows); WeDiag1 (KMAX rows)]
    H1 = DH + KMAX
    rhs1 = np.zeros((H1, KMAX * DH), np.float16)
    for d in range(DH):
        rhs1[d, d::DH] = 1.0
    for k in range(KMAX):
        rhs1[DH + k, k * DH:(k + 1) * DH] = We1
    # layer 2: rhsCat2 = [IdTile2 (32 rows); WeDiag2 (KMAX rows)], 64-col slots
    H2 = DO + KMAX
    rhs2 = np.zeros((H2, KMAX * DH), np.float16)
    for d in range(DO):
        rhs2[d, d::DH] = 1.0
    for k in range(KMAX):
        rhs2[DO + k, k * DH:k * DH + DO] = We2
    fake1 = np.where(np.arange(DH) < p1, -1e6, 1e6).astype(np.float32)[None, :]
    f2 = np.zeros(DH, np.float32)
    f2[:DO] = np.where(np.arange(DO) < p2, -1e6, 1e6)
    fake2 = f2[None, :]
    return rhs1, rhs2, fake1, fake2


# ----------------------------------------------------------------------------
# device program
# ----------------------------------------------------------------------------

def _build(plan):
    import concourse.bacc as bacc
    import concourse.bass as bass
    import concourse.mybir as mybir
    import concourse.tile as tile
    from concourse.library_config import mlp
    from concourse.masks import make_identity

    f32 = mybir.dt.float32
    f16 = mybir.dt.float16
    i16 = mybir.dt.int16
    Op = mybir.AluOpType
    Act = mybir.ActivationFunctionType

    K, KA, KB = plan["K"], plan["KA"], plan["KB"]
    KMAX, totK = plan["KMAX"], plan["totK"]
    colsA, colsB = plan["colsA"], plan["colsB"]
    P1, P2 = plan["p1"], plan["p2"]
    nA_cols = int(colsA[-1])
    nB_cols = max(int(colsB[-1]), 16)
    H1, H2 = DH + KMAX, DO + KMAX

    nc = bacc.Bacc("TRN2", debug=False, num_swdge_queues=4)

    def din(name, shape, dt=f32):
        return nc.dram_tensor(name, shape, dt, kind="ExternalInput")

    xT_d = din("xT", [D_IN, NLP])
    idxA_d = din("idxA", [128, nA_cols], i16)
    idxB_d = din("idxB", [128, nB_cols], i16)
    wT_d = din("wT", [KMAX, NLP], f16)
    rhs1_d = din("rhs1", [H1, KMAX * DH], f16)
    rhs2_d = din("rhs2", [H2, KMAX * DH], f16)
    Wl1T_d, Wr1T_d = din("Wl1T", [D_IN, DH]), din("Wr1T", [D_IN, DH])
    bl1R_d, br1_d = din("bl1R", [1, DH]), din("br1", [DH, 1])
    Wl2b_d = din("Wl2b", [DH + 1, DO], f16)   # [Wl2T; bl2 row]
    Wr2b_d = din("Wr2b", [DH + 1, DO], f16)   # [Wr2T; br2 row]
    inva1_d, bias1_d = din("inva1", [1, DH]), din("bias1", [1, DH])
    inva2_d, bias2_d = din("inva2", [1, DO]), din("bias2", [1, DO])
    fake1_d = din("fake1", [1, DH])
    fake2_d = din("fake2", [1, DH])

    out_d = nc.dram_tensor("out", [NLP, DO], f32, kind="ExternalOutput")

    bounce1 = nc.dram_tensor("bounce1", [NLF, DH], f32)
    table1 = nc.dram_tensor("table1", [NT, DH], f32)
    bounce2 = nc.dram_tensor("bounce2", [NLF, DH], f32)
    table2 = nc.dram_tensor("table2", [NT, DH], f32)

    with tile.TileContext(nc) as tc:
      with tc.tile_pool(name="persist", bufs=1) as pp:
        ident = pp.tile([P, P], f32)
        make_identity(nc, ident[:])
        nc.gpsimd.load_library(mlp)

        idxA_t = pp.tile([128, nA_cols], i16)
        idxB_t = pp.tile([128, nB_cols], i16)
        rhs1_t = pp.tile([H1, KMAX * DH], f16)
        rhs2_t = pp.tile([H2, KMAX * DH], f16)
        LT1 = pp.tile([H1, NLP], f16)     # rows 0:64 xr1T, 64:H1 wT
        LT2 = pp.tile([H2, NLP], f16)     # rows 0:32 xr2T, 32:H2 wT
        hT = pp.tile([DH + 1, NLP], f16)  # row 64 = ones (bias fold)
        Wl1T_t = pp.tile([D_IN, DH], f32, tag="Wl1T_t")
        Wr1T_t = pp.tile([D_IN, DH], f32, tag="Wr1T_t")
        bl1R_t = pp.tile([1, DH], f32, tag="bl1R_t")
        br1_t = pp.tile([DH, 1], f32, tag="br1_t")
        Wl2b_t = pp.tile([DH + 1, DO], f16, tag="Wl2b_t")
        Wr2b_t = pp.tile([DH + 1, DO], f16, tag="Wr2b_t")
        ones1_t = pp.tile([1, P], f32, tag="ones1_t")
        inva1_t = pp.tile([P, DH], f32)
        bias1_t = pp.tile([P, DH], f32)
        inva2_t = pp.tile([P, DO], f32)
        bias2_t = pp.tile([P, DO], f32)
        fake1_t = pp.tile([1, DH], f32, tag="fake1_t")
        fake2_t = pp.tile([1, DH], f32, tag="fake2_t")

        nc.vector.memset(hT[DH:DH + 1, :], 1.0)
        nc.vector.memset(ones1_t[:], 1.0)

        for t, d in [(idxA_t, idxA_d), (idxB_t, idxB_d),
                     (rhs1_t, rhs1_d), (rhs2_t, rhs2_d),
                     (Wl1T_t, Wl1T_d), (Wr1T_t, Wr1T_d),
                     (bl1R_t, bl1R_d), (br1_t, br1_d),
                     (Wl2b_t, Wl2b_d), (Wr2b_t, Wr2b_d),
                     (fake1_t, fake1_d), (fake2_t, fake2_d)]:
            nc.sync.dma_start(t[:], d[:])
        nc.sync.dma_start(LT1[DH:DH + KMAX, :], wT_d[:])
        nc.sync.dma_start(LT2[DO:DO + KMAX, :], wT_d[:])
        for t, d, dd in [(inva1_t, inva1_d, DH), (bias1_t, bias1_d, DH),
                         (inva2_t, inva2_d, DO), (bias2_t, bias2_d, DO)]:
            nc.sync.dma_start(t[:], d[:].to_broadcast([P, dd]))
        nc.sync.dma_start(bounce1[6250:6251, :], fake1_t[:])
        nc.sync.dma_start(bounce2[6250:6251, :], fake2_t[:])

        def bcast_inner(ap, n):
            return bass.AP(ap.tensor, ap.offset, [*ap.ap, [0, n]])

        # ------------------ dense phase (either layer) -------------------
        def dense(layer):
            """xl node-major -> bounce rows; xr feature-major -> LT rows."""
            if layer == 1:
                DOUT, LT, bounce = DH, LT1, bounce1
            else:
                DOUT, LT, bounce = DO, LT2, bounce2
            with (
                tc.tile_pool(name=f"dps{layer}", bufs=2, space="PSUM") as dps,
                tc.tile_pool(name=f"dsb{layer}", bufs=3) as dsb,
            ):
                for j in range(NB):
                    c0 = j * P
                    if layer == 1:
                        lhs_blk = xT_s[:, c0:c0 + P]
                        ps = dps.tile([P, DOUT], f32, tag="xl", space="PSUM")
                        nc.tensor.matmul(out=ps[:], lhsT=lhs_blk,
                                         rhs=Wl1T_t[:], start=True, stop=False)
                        nc.tensor.matmul(out=ps[:], lhsT=ones1_t[:],
                                         rhs=bl1R_t[:], start=False, stop=True,
                                         skip_group_check=True)
                        ps2 = dps.tile([DOUT, P], f32, tag="xr", space="PSUM")
                        nc.tensor.matmul(out=ps2[:], lhsT=Wr1T_t[:],
                                         rhs=lhs_blk, start=True, stop=True)
                        nc.scalar.activation(out=LT[0:DOUT, c0:c0 + P],
                                             in_=ps2[:], func=Act.Identity,
                                             bias=br1_t[:])
                    else:
                        lhs_blk = hT[:, c0:c0 + P]
                        ps = dps.tile([P, DOUT], f32, tag="xl", space="PSUM")
                        nc.tensor.matmul(out=ps[:], lhsT=lhs_blk,
                                         rhs=Wl2b_t[:], start=True, stop=True)
                        ps2 = dps.tile([DOUT, P], f32, tag="xr", space="PSUM")
                        nc.tensor.matmul(out=ps2[:], lhsT=Wr2b_t[:],
                                         rhs=lhs_blk, start=True, stop=True)
                        nc.scalar.activation(out=LT[0:DOUT, c0:c0 + P],
                                             in_=ps2[:], func=Act.Identity,
                                             bias=0.0)
                    st = dsb.tile([P, DH], f32, tag="st")
                    if layer == 2:
                        nc.vector.memset(st[:], 0.0)
                    nc.scalar.activation(out=st[:, 0:DOUT], in_=ps[:],
                                         func=Act.Identity, bias=0.0)
                    lo, hi = j * P, min((j + 1) * P, NL)
                    if hi > lo:
                        nc.sync.dma_start(out=bounce[lo:hi, :],
                                          in_=st[:hi - lo, :])

        # ------------------ edge phase (either layer) --------------------
        qctr = [0]

        def edge(layer):
            if layer == 1:
                D, p1, table, LT, rhs_t, H = DH, P1, table1, LT1, rhs1_t, H1
                inva_rep, bias_rep = inva1_t, bias1_t
            else:
                D, p1, table, LT, rhs_t, H = DO, P2, table2, LT2, rhs2_t, H2
                inva_rep, bias_rep = inva2_t, bias2_t

            def do_block(j, pv, sg, sb, sm, ph):
                Kj, KAj, KBj = int(K[j]), int(KA[j]), int(KB[j])
                W = Kj * DH
                g_t = sg.tile([P, KMAX, DH], f32, tag="g")
                q = GQ[qctr[0] % len(GQ)]
                qctr[0] += 1
                nc.gpsimd.dma_gather(
                    g_t[:, 0:KAj, :], table[0:SPLIT, :],
                    idxA_t[:, int(colsA[j]):int(colsA[j + 1])],
                    P * KAj, P * KAj, DH, single_packet=False, queue_num=q)
                if KBj:
                    q = GQ[qctr[0] % len(GQ)]
                    qctr[0] += 1
                    nc.gpsimd.dma_gather(
                        g_t[:, KAj:Kj, :], table[SPLIT:NT, :],
                        idxB_t[:, int(colsB[j]):int(colsB[j + 1])],
                        P * KBj, P * KBj, DH, single_packet=False, queue_num=q)
                g_flat = g_t[:].rearrange("p k d -> p (k d)")
                lhs_blk = LT[:, j * P:(j + 1) * P]
                m_t = sb.tile([P, W], f16, tag="m")
                for g0 in range(0, W, 512):
                    g1 = min(g0 + 512, W)
                    ps_v = pv.tile([P, 512], f32, tag="v", space="PSUM")
                    nc.tensor.matmul(out=ps_v[:, 0:g1 - g0], lhsT=lhs_blk,
                                     rhs=rhs_t[:, g0:g1],
                                     start=True, stop=False)
                    nc.tensor.matmul(out=ps_v[:, 0:g1 - g0], lhsT=ident[:],
                                     rhs=g_flat[:, g0:g1],
                                     start=False, stop=True)
                    nc.scalar.activation(out=m_t[:, g0:g1],
                                         in_=ps_v[:, 0:g1 - g0],
                                         func=Act.Prelu, alpha=0.2)
                m3d = m_t[:].rearrange("p (k d) -> p k d", d=DH)
                sp_t = sm.tile([P, KMAX], f32, tag="sp")
                sn_t = sm.tile([P, KMAX], f32, tag="sn")
                s_t = sm.tile([P, KMAX], f32, tag="s")
                if 0 < p1:
                    nc.vector.tensor_reduce(out=sp_t[:, 0:Kj],
                                            in_=m3d[:, :, 0:p1],
                                            axis=mybir.AxisListType.X,
                                            op=Op.add)
                if p1 < D:
                    nc.vector.tensor_reduce(out=sn_t[:, 0:Kj],
                                            in_=m3d[:, :, p1:D],
                                            axis=mybir.AxisListType.X,
                                            op=Op.add)
                if 0 < p1 < D:
                    nc.vector.scalar_tensor_tensor(
                        out=s_t[:, 0:Kj], in0=sn_t[:, 0:Kj], scalar=-1.0,
                        in1=sp_t[:, 0:Kj], op0=Op.mult, op1=Op.add)
                elif p1 == D:
                    s_t = sp_t
                else:
                    nc.vector.tensor_scalar(out=s_t[:, 0:Kj],
                                            in0=sn_t[:, 0:Kj],
                                            scalar1=-1.0, scalar2=None,
                                            op0=Op.mult)
                e_t = sm.tile([P, KMAX], f32, tag="e")
                Z_t = sm.tile([P, 1], f32, tag="Z")
                nc.scalar.activation(out=e_t[:, 0:Kj], in_=s_t[:, 0:Kj],
                                     func=Act.Exp, accum_out=Z_t[:])
                iZ_t = sm.tile([P, 1], f32, tag="iZ")
                nc.vector.reciprocal(out=iZ_t[:], in_=Z_t[:])
                # m is dead after sp/sn: reuse its space for e*g (SBUF budget)
                nc.vector.tensor_tensor(out=m_t[:], in0=g_flat[:, 0:W],
                                        in1=bcast_inner(e_t[:, 0:Kj], DH),
                                        op=Op.mult)
                agg_t = sm.tile([P, DH], f32, tag="agg")
                nc.vector.tensor_reduce(
                    out=agg_t[:],
                    in_=m_t[:].rearrange("p (k d) -> p d k", d=DH),
                    axis=mybir.AxisListType.X, op=Op.add)
                t4_t = sm.tile([P, D], f32, tag="t4")
                nc.vector.tensor_tensor(out=t4_t[:], in0=agg_t[:, 0:D],
                                        in1=inva_rep[:], op=Op.mult)
                t5_t = sm.tile([P, D], f32, tag="t5")
                nc.vector.scalar_tensor_tensor(
                    out=t5_t[:], in0=t4_t[:], scalar=iZ_t[:],
                    in1=bias_rep[:], op0=Op.mult, op1=Op.add)
                if layer == 1:
                    u1_t = sm.tile([P, D], f32, tag="u1")
                    nc.vector.tensor_scalar(out=u1_t[:], in0=t5_t[:],
                                            scalar1=0.0, scalar2=None,
                                            op0=Op.min)
                    u2_t = sm.tile([P, D], f32, tag="u2")
                    nc.scalar.activation(out=u2_t[:], in_=u1_t[:],
                                         func=Act.Exp)
                    h_t = sm.tile([P, D], f32, tag="h")
                    nc.vector.scalar_tensor_tensor(
                        out=h_t[:], in0=t5_t[:], scalar=0.0, in1=u2_t[:],
                        op0=Op.max, op1=Op.add)
                    pst = ph.tile([DH, P], f32, tag="ht", space="PSUM")
                    nc.tensor.transpose(out=pst[:], in_=h_t[:],
                                        identity=ident[:])
                    nc.scalar.activation(out=hT[0:DH, j * P:(j + 1) * P],
                                         in_=pst[:], func=Act.Identity,
                                         bias=0.0)
                else:
                    # softplus(z) = relu(z) + ln(1 + exp(-|z|))
                    ab_t = sm.tile([P, D], f32, tag="ab")
                    nc.scalar.activation(out=ab_t[:], in_=t5_t[:],
                                         func=Act.Abs)
                    ex_t = sm.tile([P, D], f32, tag="ex")
                    nc.scalar.activation(out=ex_t[:], in_=ab_t[:],
                                         func=Act.Exp, scale=-1.0)
                    ln_t = sm.tile([P, D], f32, tag="ln")
                    nc.scalar.activation(out=ln_t[:], in_=ex_t[:],
                                         func=Act.Ln, bias=1.0)
                    o_t = sm.tile([P, D], f32, tag="o")
                    nc.vector.scalar_tensor_tensor(
                        out=o_t[:], in0=t5_t[:], scalar=0.0, in1=ln_t[:],
                        op0=Op.max, op1=Op.add)
                    nc.sync.dma_start(out=out_d[j * P:(j + 1) * P, :],
                                      in_=o_t[:])

            with (
                tc.tile_pool(name=f"pv{layer}", bufs=6, space="PSUM") as pv,
                tc.tile_pool(name=f"ph{layer}", bufs=2, space="PSUM") as ph,
                tc.tile_pool(name=f"sg{layer}", bufs=6) as sg,
                tc.tile_pool(name=f"sb{layer}", bufs=7) as sb,
                tc.tile_pool(name=f"sm{layer}", bufs=6) as sm,
            ):
                for j in range(NB):
                    do_block(j, pv, sg, sb, sm, ph)

        # ---------------------------- schedule ---------------------------
        with tc.tile_pool(name="xt", bufs=1) as xtp:
            xT_s = xtp.tile([D_IN, NLP], f32)
            nc.sync.dma_start(xT_s[:], xT_d[:])
            dense(1)

        nc.gpsimd.collective_compute(
            "AllGather", Op.bypass, replica_groups=[list(range(C))],
            ins=[bounce1[:]], outs=[table1[:]])

        edge(1)
        dense(2)

        nc.gpsimd.collective_compute(
            "AllGather", Op.bypass, replica_groups=[list(range(C))],
            ins=[bounce2[:]], outs=[table2[:]])

        edge(2)

    nc.compile()
    return nc


# ----------------------------------------------------------------------------
# entry point
# ----------------------------------------------------------------------------

def _make_in_maps(inputs):
    x = np.asarray(inputs["x"], np.float32)
    f1 = _fold(inputs["Wl1"], inputs["bl1"], inputs["Wr1"], inputs["br1"],
               inputs["We1"], inputs["att1"], inputs["bias1"])
    f2 = _fold(inputs["Wl2"], inputs["bl2"], inputs["Wr2"], inputs["br2"],
               inputs["We2"], inputs["att2"], inputs["bias2"],
               in_perm=f1["pi"], h_offset=True)
    g = _prep(x, inputs["edge_index"], inputs["edge_weight"])
    rhs1, rhs2, fake1, fake2 = _consts(g["KMAX"], f1["We"], f2["We"],
                                       f1["p1"], f2["p1"])

    plan = dict(g, p1=f1["p1"], p2=f2["p1"])
    Wl2b = np.concatenate([f2["WlT"], f2["blRow"]], 0).astype(np.float16)
    Wr2b = np.concatenate([f2["WrT"], f2["brRow"]], 0).astype(np.float16)
    shared = dict(
        Wl1T=f1["WlT"], Wr1T=f1["WrT"], bl1R=f1["blRow"], br1=f1["br"],
        Wl2b=Wl2b, Wr2b=Wr2b,
        rhs1=rhs1, rhs2=rhs2, fake1=fake1, fake2=fake2,
        inva1=f1["inva"][None, :], bias1=f1["bias"][None, :],
        inva2=f2["inva"][None, :], bias2=f2["bias"][None, :],
    )
    in_maps = []
    for c in range(C):
        m = dict(shared)
        m.update(xT=g["xT"][c], idxA=g["idxA"][c], idxB=g["idxB"][c],
                 wT=g["wT"][c])
        in_maps.append(m)
    return plan, in_maps, g, f2


def kernel(**inputs):
    from concourse.bass_utils import run_bass_kernel_spmd

    plan, in_maps, g, f2 = _make_in_maps(inputs)
    nc = _build(plan)
    res = run_bass_kernel_spmd(nc, in_maps, list(range(C)))

    full_new = np.concatenate([res.results[c]["out"][:NL] for c in range(C)], 0)
    full_old = full_new[g["new_id"]]
    out = np.empty((N, DO), np.float32)
    out[:, f2["pi"]] = full_old + 1e-4
    return out.astype(np.float32)


# revision 13
# speedup vs baseline: 1.2105x; 1.2105x over previous
"""Two-layer GATv2 (heads=1, edge_dim=1) on 8 Trainium2 NeuronCores.

Sharding: nodes dealt round-robin by in-degree onto 8 cores; dst-grouped
edges stay local; source features come from an AllGather'd full table via
dma_gather on three concurrent SWDGE queues (queues 1-3 dispatch in ~0.4us
and generate descriptors on their own Q7 core pairs in parallel).

Per 128-node (dst) block:
  v   = [xrT | w]-lhsT @ [IdTile | WeDiag] + ident @ g    -- 2 matmuls/chunk
  m   = prelu(v, 0.2)                                     -- ACT (from PSUM)
  s   = sum_{d<P1} m - sum_{d>=P1} m                      -- att sign-split
  e   = exp(s), Z = row-sum(e)                            -- ACT with accum
  agg = sum_k e_k * g_k                                   -- DVE mult+reduce
  out = agg * inva * (1/Z) + bias                         -- undo |att| scale
Padded gather slots point at a per-core "fake" table row holding -/+1e6 in
the sign-sorted layout, so exp(s)==0 exactly -- no mask tensors needed.
Dense phase computes xl node-major (lhsT = xT block) and xr feature-major
(lhsT = WrT) directly -- no PE transposes; biases fold into rank-1 matmul /
ACT bias. Layer-2 softplus runs on ACT; the +1e-4 is added host-side.
"""

import numpy as np

N, E, D_IN, DH, DO = 50000, 800000, 128, 64, 32
C = 8                      # cores
NL = N // C                # nodes per core (6250)
P = 128                    # partitions = nodes per block
NB = (NL + P - 1) // P     # blocks per core (49)
NLP = NB * P               # padded nodes per core (6272)
NLF = 6256                 # bounce rows per core (6250 real + fake + pad)
SPLIT = 32768              # int16 gather table split (table-row space)
NT = C * NLF               # table rows (50048)
VSMALL = 32                # K threshold for double-buffered PSUM v-tiles
FAKE_A = 6250              # core-0 fake row (side A)
FAKE_B = 5 * NLF + 6250 - SPLIT   # core-5 fake row offset in side B
GQ = (1, 2, 3, 0)          # SWDGE queues (0 last: it blocks the sequencer)


# ----------------------------------------------------------------------------
# host-side: weight folding and graph layout
# ----------------------------------------------------------------------------

def _fold(Wl, bl, Wr, br, We, att, bias, in_perm=None, h_offset=False):
    att = np.asarray(att, np.float64)
    pi = np.concatenate([np.nonzero(att >= 0)[0], np.nonzero(att < 0)[0]])
    p1 = int((att >= 0).sum())
    a = np.maximum(np.abs(att[pi]), 1e-30)
    Wl = np.asarray(Wl, np.float64)[pi] * a[:, None]
    Wr = np.asarray(Wr, np.float64)[pi] * a[:, None]
    bl = np.asarray(bl, np.float64)[pi] * a
    br = np.asarray(br, np.float64)[pi] * a
    We_ = np.asarray(We, np.float64)[pi, 0] * a
    if in_perm is not None:
        Wl = Wl[:, in_perm]
        Wr = Wr[:, in_perm]
    if h_offset:  # input arrives as h+1
        bl = bl - Wl.sum(1)
        br = br - Wr.sum(1)
    return dict(
        WlT=np.ascontiguousarray(Wl.T, np.float32),
        WrT=np.ascontiguousarray(Wr.T, np.float32),
        blRow=bl.astype(np.float32)[None, :],
        br=br.astype(np.float32)[:, None],
        brRow=br.astype(np.float32)[None, :],
        We=We_.astype(np.float32),
        inva=(1.0 / a).astype(np.float32),
        bias=np.asarray(bias, np.float64)[pi].astype(np.float32),
        pi=pi, p1=p1,
    )


def _prep(x, edge_index, edge_weight):
    src = np.asarray(edge_index[0], np.int64)
    dst = np.asarray(edge_index[1], np.int64)
    w = np.asarray(edge_weight, np.float32)

    deg = np.bincount(dst, minlength=N)
    wsum = np.bincount(dst, weights=w.astype(np.float64), minlength=N)
    loop_w = (wsum / np.maximum(deg, 1)).astype(np.float32)

    order = np.argsort(-deg, kind="stable")
    ranks = np.empty(N, np.int64)
    ranks[order] = np.arange(N)
    core = ranks % C
    crank = ranks // C
    new_id = core * NL + crank          # output-row space
    trow = core * NLF + crank           # gather-table-row space
    inv = np.empty(N, np.int64)
    inv[new_id] = np.arange(N)          # old id of each new id

    esrc = np.concatenate([trow[src], trow])          # table rows of sources
    edst = np.concatenate([new_id[dst], new_id])      # local ids of dsts
    ew = np.concatenate([w, loop_w]).astype(np.float32)

    side = (esrc >= SPLIT).astype(np.int64)
    eord = np.argsort(edst * 2 + side, kind="stable")
    sdst, ssrc, sw, sside = edst[eord], esrc[eord], ew[eord], side[eord]

    nA = np.bincount(edst[side == 0], minlength=N)
    nB = np.bincount(edst[side == 1], minlength=N)

    grp = np.searchsorted(sdst, np.arange(N))
    pos = np.arange(E + N) - grp[sdst]
    posAB = np.where(sside == 0, pos, pos - nA[sdst])

    nblk = (np.arange(N) % NL) // P      # block of each new id
    KA = np.zeros(NB, np.int64)
    KB = np.zeros(NB, np.int64)
    np.maximum.at(KA, nblk, nA)
    np.maximum.at(KB, nblk, nB)
    KA = np.maximum(KA, 1)
    K = KA + KB
    assert int(K.max()) * DH * 4 <= 16384, f"KMAX {K.max()} overflows PSUM"
    KMAX = int(K.max())
    totK = int(K.sum())

    e_core = sdst // NL
    e_loc = sdst % NL
    e_blk = e_loc // P
    e_p = e_loc % P
    e_k = np.where(sside == 0, posAB, KA[e_blk] + posAB)

    colsA = np.concatenate([[0], np.cumsum(KA * 8)]).astype(np.int64)
    colsB = np.concatenate([[0], np.cumsum(KB * 8)]).astype(np.int64)
    idxA = np.full((C, 128, int(colsA[-1])), FAKE_A, np.int16)
    idxB = np.full((C, 128, max(int(colsB[-1]), 16)), FAKE_B, np.int16)
    mA = sside == 0
    fA = e_k[mA] * P + e_p[mA]
    idxA[e_core[mA], fA % 16, colsA[e_blk[mA]] + fA // 16] = \
        ssrc[mA].astype(np.int16)
    mB = ~mA
    fB = (e_k[mB] - KA[e_blk[mB]]) * P + e_p[mB]
    idxB[e_core[mB], fB % 16, colsB[e_blk[mB]] + fB // 16] = \
        (ssrc[mB] - SPLIT).astype(np.int16)
    if NL % P:  # dummy partitions in last block: avoid Z=0 (point at row 0)
        for p in range(NL % P, P):
            f = 0 * P + p
            idxA[:, f % 16, colsA[NB - 1] + f // 16] = 0
    for rep in range(1, 8):
        idxA[:, 16 * rep:16 * rep + 16] = idxA[:, :16]
        idxB[:, 16 * rep:16 * rep + 16] = idxB[:, :16]

    x = np.asarray(x, np.float32)
    xT = np.zeros((C, D_IN, NLP), np.float32)
    perm = inv.reshape(C, NL)
    for c in range(C):
        xT[c, :, :NL] = x[perm[c]].T

    wT = np.zeros((C, KMAX, NLP), np.float16)
    wT[e_core, e_k, e_blk * P + e_p] = sw.astype(np.float16)

    return dict(new_id=new_id, K=K, KA=KA, KB=KB, KMAX=KMAX, totK=totK,
                colsA=colsA, colsB=colsB, idxA=idxA, idxB=idxB, xT=xT, wT=wT)


def _consts(KMAX, We1, We2, p1, p2):
    # layer 1: rhsCat1 = [IdTile1 (64 rows); WeDiag1 (KMAX rows)]
    H1 = DH + KMAX
    rhs1 = np.zeros((H1, KMAX * DH), np.float16)
    for d in range(DH):
        rhs1[d, d::DH] = 1.0
    for k in range(KMAX):
        rhs1[DH + k, k * DH:(k + 1) * DH] = We1
    # layer 2: rhsCat2 = [IdTile2 (32 rows); WeDiag2 (KMAX rows)], 64-col slots
    H2 = DO + KMAX
    rhs2 = np.zeros((H2, KMAX * DH), np.float16)
    for d in range(DO):
        rhs2[d, d::DH] = 1.0
    for k in range(KMAX):
        rhs2[DO + k, k * DH:k * DH + DO] = We2
    fake1 = np.where(np.arange(DH) < p1, -1e6, 1e6).astype(np.float32)[None, :]
    f2 = np.zeros(DH, np.float32)
    f2[:DO] = np.where(np.arange(DO) < p2, -1e6, 1e6)
    fake2 = f2[None, :]
    return rhs1, rhs2, fake1, fake2


# ----------------------------------------------------------------------------
# device program
# ----------------------------------------------------------------------------

def _build(plan):
    import concourse.bacc as bacc
    import concourse.bass as bass
    import concourse.mybir as mybir
    import concourse.tile as tile
    from concourse.library_config import mlp
    from concourse.masks import make_identity

    f32 = mybir.dt.float32
    f16 = mybir.dt.float16
    i16 = mybir.dt.int16
    Op = mybir.AluOpType
    Act = mybir.ActivationFunctionType

    K, KA, KB = plan["K"], plan["KA"], plan["KB"]
    KMAX, totK = plan["KMAX"], plan["totK"]
    colsA, colsB = plan["colsA"], plan["colsB"]
    P1, P2 = plan["p1"], plan["p2"]
    nA_cols = int(colsA[-1])
    nB_cols = max(int(colsB[-1]), 16)
    H1, H2 = DH + KMAX, DO + KMAX

    nc = bacc.Bacc("TRN2", debug=False, num_swdge_queues=4)

    def din(name, shape, dt=f32):
        return nc.dram_tensor(name, shape, dt, kind="ExternalInput")

    xT_d = din("xT", [D_IN, NLP])
    idxA_d = din("idxA", [128, nA_cols], i16)
    idxB_d = din("idxB", [128, nB_cols], i16)
    wT_d = din("wT", [KMAX, NLP], f16)
    rhs1_d = din("rhs1", [H1, KMAX * DH], f16)
    rhs2_d = din("rhs2", [H2, KMAX * DH], f16)
    Wl1T_d, Wr1T_d = din("Wl1T", [D_IN, DH]), din("Wr1T", [D_IN, DH])
    bl1R_d, br1_d = din("bl1R", [1, DH]), din("br1", [DH, 1])
    Wl2b_d = din("Wl2b", [DH + 1, DO], f16)   # [Wl2T; bl2 row]
    Wr2b_d = din("Wr2b", [DH + 1, DO], f16)   # [Wr2T; br2 row]
    inva1_d, bias1_d = din("inva1", [1, DH]), din("bias1", [1, DH])
    inva2_d, bias2_d = din("inva2", [1, DO]), din("bias2", [1, DO])
    fake1_d = din("fake1", [1, DH])
    fake2_d = din("fake2", [1, DH])

    out_d = nc.dram_tensor("out", [NLP, DO], f32, kind="ExternalOutput")

    bounce1 = nc.dram_tensor("bounce1", [NLF, DH], f32)
    table1 = nc.dram_tensor("table1", [NT, DH], f32)
    bounce2 = nc.dram_tensor("bounce2", [NLF, DH], f32)
    table2 = nc.dram_tensor("table2", [NT, DH], f32)

    with tile.TileContext(nc) as tc:
      with tc.tile_pool(name="persist", bufs=1) as pp:
        ident = pp.tile([P, P], f32)
        make_identity(nc, ident[:])
        nc.gpsimd.load_library(mlp)

        idxA_t = pp.tile([128, nA_cols], i16)
        idxB_t = pp.tile([128, nB_cols], i16)
        rhs1_t = pp.tile([H1, KMAX * DH], f16)
        rhs2_t = pp.tile([H2, KMAX * DH], f16)
        LT1 = pp.tile([H1, NLP], f16)     # rows 0:64 xr1T, 64:H1 wT
        LT2 = pp.tile([H2, NLP], f16)     # rows 0:32 xr2T, 32:H2 wT
        hT = pp.tile([DH + 1, NLP], f16)  # row 64 = ones (bias fold)
        Wl1T_t = pp.tile([D_IN, DH], f32, tag="Wl1T_t")
        Wr1T_t = pp.tile([D_IN, DH], f32, tag="Wr1T_t")
        bl1R_t = pp.tile([1, DH], f32, tag="bl1R_t")
        br1_t = pp.tile([DH, 1], f32, tag="br1_t")
        Wl2b_t = pp.tile([DH + 1, DO], f16, tag="Wl2b_t")
        Wr2b_t = pp.tile([DH + 1, DO], f16, tag="Wr2b_t")
        ones1_t = pp.tile([1, P], f32, tag="ones1_t")
        inva1_t = pp.tile([P, DH], f32)
        bias1_t = pp.tile([P, DH], f32)
        inva2_t = pp.tile([P, DO], f32)
        bias2_t = pp.tile([P, DO], f32)
        fake1_t = pp.tile([1, DH], f32, tag="fake1_t")
        fake2_t = pp.tile([1, DH], f32, tag="fake2_t")

        nc.vector.memset(hT[DH:DH + 1, :], 1.0)
        nc.vector.memset(ones1_t[:], 1.0)

        for t, d in [(idxA_t, idxA_d), (idxB_t, idxB_d),
                     (rhs1_t, rhs1_d), (rhs2_t, rhs2_d),
                     (Wl1T_t, Wl1T_d), (Wr1T_t, Wr1T_d),
                     (bl1R_t, bl1R_d), (br1_t, br1_d),
                     (Wl2b_t, Wl2b_d), (Wr2b_t, Wr2b_d),
                     (fake1_t, fake1_d), (fake2_t, fake2_d)]:
            nc.sync.dma_start(t[:], d[:])
        nc.sync.dma_start(LT1[DH:DH + KMAX, :], wT_d[:])
        nc.sync.dma_start(LT2[DO:DO + KMAX, :], wT_d[:])
        for t, d, dd in [(inva1_t, inva1_d, DH), (bias1_t, bias1_d, DH),
                         (inva2_t, inva2_d, DO), (bias2_t, bias2_d, DO)]:
            nc.sync.dma_start(t[:], d[:].to_broadcast([P, dd]))
        nc.sync.dma_start(bounce1[6250:6251, :], fake1_t[:])
        nc.sync.dma_start(bounce2[6250:6251, :], fake2_t[:])

        def bcast_inner(ap, n):
            return bass.AP(ap.tensor, ap.offset, [*ap.ap, [0, n]])

        # ------------------ dense phase (either layer) -------------------
        def dense(layer):
            """xl node-major -> bounce rows; xr feature-major -> LT rows."""
            if layer == 1:
                DOUT, LT, bounce = DH, LT1, bounce1
            else:
                DOUT, LT, bounce = DO, LT2, bounce2
            with (
                tc.tile_pool(name=f"dps{layer}", bufs=2, space="PSUM") as dps,
                tc.tile_pool(name=f"dsb{layer}", bufs=3) as dsb,
            ):
                for j in range(NB):
                    c0 = j * P
                    if layer == 1:
                        lhs_blk = xT_s[:, c0:c0 + P]
                        ps = dps.tile([P, DOUT], f32, tag="xl", space="PSUM")
                        nc.tensor.matmul(out=ps[:], lhsT=lhs_blk,
                                         rhs=Wl1T_t[:], start=True, stop=False)
                        nc.tensor.matmul(out=ps[:], lhsT=ones1_t[:],
                                         rhs=bl1R_t[:], start=False, stop=True,
                                         skip_group_check=True)
                        ps2 = dps.tile([DOUT, P], f32, tag="xr", space="PSUM")
                        nc.tensor.matmul(out=ps2[:], lhsT=Wr1T_t[:],
                                         rhs=lhs_blk, start=True, stop=True)
                        nc.scalar.activation(out=LT[0:DOUT, c0:c0 + P],
                                             in_=ps2[:], func=Act.Identity,
                                             bias=br1_t[:])
                    else:
                        lhs_blk = hT[:, c0:c0 + P]
                        ps = dps.tile([P, DOUT], f32, tag="xl", space="PSUM")
                        nc.tensor.matmul(out=ps[:], lhsT=lhs_blk,
                                         rhs=Wl2b_t[:], start=True, stop=True)
                        ps2 = dps.tile([DOUT, P], f32, tag="xr", space="PSUM")
                        nc.tensor.matmul(out=ps2[:], lhsT=Wr2b_t[:],
                                         rhs=lhs_blk, start=True, stop=True)
                        nc.scalar.activation(out=LT[0:DOUT, c0:c0 + P],
                                             in_=ps2[:], func=Act.Identity,
                                             bias=0.0)
                    st = dsb.tile([P, DH], f32, tag="st")
                    if layer == 2:
                        nc.vector.memset(st[:], 0.0)
                    nc.scalar.activation(out=st[:, 0:DOUT], in_=ps[:],
                                         func=Act.Identity, bias=0.0)
                    lo, hi = j * P, min((j + 1) * P, NL)
                    if hi > lo:
                        nc.sync.dma_start(out=bounce[lo:hi, :],
                                          in_=st[:hi - lo, :])

        # ------------------ edge phase (either layer) --------------------
        qctr = [0]

        def edge(layer):
            if layer == 1:
                D, p1, table, LT, rhs_t, H = DH, P1, table1, LT1, rhs1_t, H1
                inva_rep, bias_rep = inva1_t, bias1_t
            else:
                D, p1, table, LT, rhs_t, H = DO, P2, table2, LT2, rhs2_t, H2
                inva_rep, bias_rep = inva2_t, bias2_t

            S = {}  # per-block tile state across pipeline stages

            def st_gather(j, sg):
                Kj, KAj, KBj = int(K[j]), int(KA[j]), int(KB[j])
                g_t = sg.tile([P, KMAX, DH], f32, tag="g")
                q = GQ[qctr[0] % len(GQ)]
                qctr[0] += 1
                nc.gpsimd.dma_gather(
                    g_t[:, 0:KAj, :], table[0:SPLIT, :],
                    idxA_t[:, int(colsA[j]):int(colsA[j + 1])],
                    P * KAj, P * KAj, DH, single_packet=False, queue_num=q)
                if KBj:
                    q = GQ[qctr[0] % len(GQ)]
                    qctr[0] += 1
                    nc.gpsimd.dma_gather(
                        g_t[:, KAj:Kj, :], table[SPLIT:NT, :],
                        idxB_t[:, int(colsB[j]):int(colsB[j + 1])],
                        P * KBj, P * KBj, DH, single_packet=False, queue_num=q)
                S[j] = dict(g=g_t)

            def st_mm(j, pv, sb):
                Kj = int(K[j])
                W = Kj * DH
                g_flat = S[j]["g"][:].rearrange("p k d -> p (k d)")
                lhs_blk = LT[:, j * P:(j + 1) * P]
                m_t = sb.tile([P, W], f16, tag="m")
                for g0 in range(0, W, 512):
                    g1 = min(g0 + 512, W)
                    ps_v = pv.tile([P, 512], f32, tag="v", space="PSUM")
                    nc.tensor.matmul(out=ps_v[:, 0:g1 - g0], lhsT=lhs_blk,
                                     rhs=rhs_t[:, g0:g1],
                                     start=True, stop=False)
                    nc.tensor.matmul(out=ps_v[:, 0:g1 - g0], lhsT=ident[:],
                                     rhs=g_flat[:, g0:g1],
                                     start=False, stop=True)
                    nc.scalar.activation(out=m_t[:, g0:g1],
                                         in_=ps_v[:, 0:g1 - g0],
                                         func=Act.Prelu, alpha=0.2)
                S[j]["m"] = m_t

            def st_score(j, sm):
                Kj = int(K[j])
                m3d = S[j]["m"][:].rearrange("p (k d) -> p k d", d=DH)
                sp_t = sm.tile([P, KMAX], f32, tag="sp")
                sn_t = sm.tile([P, KMAX], f32, tag="sn")
                s_t = sm.tile([P, KMAX], f32, tag="s")
                if 0 < p1:
                    nc.vector.tensor_reduce(out=sp_t[:, 0:Kj],
                                            in_=m3d[:, :, 0:p1],
                                            axis=mybir.AxisListType.X,
                                            op=Op.add)
                if p1 < D:
                    nc.vector.tensor_reduce(out=sn_t[:, 0:Kj],
                                            in_=m3d[:, :, p1:D],
                                            axis=mybir.AxisListType.X,
                                            op=Op.add)
                if 0 < p1 < D:
                    nc.vector.scalar_tensor_tensor(
                        out=s_t[:, 0:Kj], in0=sn_t[:, 0:Kj], scalar=-1.0,
                        in1=sp_t[:, 0:Kj], op0=Op.mult, op1=Op.add)
                elif p1 == D:
                    s_t = sp_t
                else:
                    nc.vector.tensor_scalar(out=s_t[:, 0:Kj],
                                            in0=sn_t[:, 0:Kj],
                                            scalar1=-1.0, scalar2=None,
                                            op0=Op.mult)
                S[j]["s"] = s_t

            def st_exp(j, sm):
                Kj = int(K[j])
                e_t = sm.tile([P, KMAX], f32, tag="e")
                Z_t = sm.tile([P, 1], f32, tag="Z")
                nc.scalar.activation(out=e_t[:, 0:Kj], in_=S[j]["s"][:, 0:Kj],
                                     func=Act.Exp, accum_out=Z_t[:])
                S[j]["e"] = e_t
                S[j]["Z"] = Z_t

            def st_agg(j, sm):
                Kj = int(K[j])
                W = Kj * DH
                g_flat = S[j]["g"][:].rearrange("p k d -> p (k d)")
                m_t = S[j]["m"]
                iZ_t = sm.tile([P, 1], f32, tag="iZ")
                nc.vector.reciprocal(out=iZ_t[:], in_=S[j]["Z"][:])
                # m is dead after sp/sn: reuse its space for e*g
                nc.vector.tensor_tensor(out=m_t[:], in0=g_flat[:, 0:W],
                                        in1=bcast_inner(S[j]["e"][:, 0:Kj],
                                                        DH),
                                        op=Op.mult)
                agg_t = sm.tile([P, DH], f32, tag="agg")
                nc.vector.tensor_reduce(
                    out=agg_t[:],
                    in_=m_t[:].rearrange("p (k d) -> p d k", d=DH),
                    axis=mybir.AxisListType.X, op=Op.add)
                t4_t = sm.tile([P, D], f32, tag="t4")
                nc.vector.tensor_tensor(out=t4_t[:], in0=agg_t[:, 0:D],
                                        in1=inva_rep[:], op=Op.mult)
                t5_t = sm.tile([P, D], f32, tag="t5")
                nc.vector.scalar_tensor_tensor(
                    out=t5_t[:], in0=t4_t[:], scalar=iZ_t[:],
                    in1=bias_rep[:], op0=Op.mult, op1=Op.add)
                S[j]["t5"] = t5_t

            def st_act4(j, sm):
                t5_t = S[j]["t5"]
                if layer == 1:
                    # elu(x)+1 = min(exp(x), 1) + max(x, 0)
                    E_t = sm.tile([P, D], f32, tag="E")
                    nc.scalar.activation(out=E_t[:], in_=t5_t[:],
                                         func=Act.Exp)
                    r_t = sm.tile([P, D], f32, tag="r")
                    nc.vector.tensor_scalar(out=r_t[:], in0=t5_t[:],
                                            scalar1=0.0, scalar2=None,
                                            op0=Op.max)
                    S[j]["E"], S[j]["r"] = E_t, r_t
                else:
                    # softplus(z) = relu(z) + ln(1 + exp(-|z|))
                    ab_t = sm.tile([P, D], f32, tag="ab")
                    nc.scalar.activation(out=ab_t[:], in_=t5_t[:],
                                         func=Act.Abs)
                    ex_t = sm.tile([P, D], f32, tag="ex")
                    nc.scalar.activation(out=ex_t[:], in_=ab_t[:],
                                         func=Act.Exp, scale=-1.0)
                    ln_t = sm.tile([P, D], f32, tag="ln")
                    nc.scalar.activation(out=ln_t[:], in_=ex_t[:],
                                         func=Act.Ln, bias=1.0)
                    S[j]["ln"] = ln_t

            def st_fin(j, sm):
                t5_t = S[j]["t5"]
                if layer == 1:
                    h_t = sm.tile([P, D], f32, tag="h")
                    nc.vector.scalar_tensor_tensor(
                        out=h_t[:], in0=S[j]["E"][:], scalar=1.0,
                        in1=S[j]["r"][:], op0=Op.min, op1=Op.add)
                    S[j]["h"] = h_t
                else:
                    o_t = sm.tile([P, D], f32, tag="o")
                    nc.vector.scalar_tensor_tensor(
                        out=o_t[:], in0=t5_t[:], scalar=0.0,
                        in1=S[j]["ln"][:], op0=Op.max, op1=Op.add)
                    nc.sync.dma_start(out=out_d[j * P:(j + 1) * P, :],
                                      in_=o_t[:])

            def st_tr(j, ph):
                if layer == 1:
                    pst = ph.tile([DH, P], f32, tag="ht", space="PSUM")
                    nc.tensor.transpose(out=pst[:], in_=S[j]["h"][:],
                                        identity=ident[:])
                    S[j]["pst"] = pst

            def st_copy(j):
                if layer == 1:
                    nc.scalar.activation(out=hT[0:DH, j * P:(j + 1) * P],
                                         in_=S[j]["pst"][:],
                                         func=Act.Identity, bias=0.0)
                S.pop(j)

            # lags relative to the loop index (stage runs on block i - lag)
            stages = [
                (4, st_exp),     # ACT, before prelus in ACT stream
                (7, st_act4),    # ACT
                (10, st_copy),   # ACT                (layer 1)
                (5, st_agg),     # DVE, before scores in DVE stream
                (8, st_fin),     # DVE (+ out DMA on layer 2)
                (9, st_tr),      # PE, before mms     (layer 1)
                (0, st_gather),  # GpSimd prefetch
                (3, st_mm),      # PE + ACT prelus
                (3, st_score),   # DVE
            ]
            with (
                tc.tile_pool(name=f"pv{layer}", bufs=5, space="PSUM") as pv,
                tc.tile_pool(name=f"ph{layer}", bufs=3, space="PSUM") as ph,
                tc.tile_pool(name=f"sg{layer}", bufs=6) as sg,
                tc.tile_pool(name=f"sb{layer}", bufs=7) as sb,
                tc.tile_pool(name=f"sm{layer}", bufs=6) as sm,
            ):
                args = {st_gather: (sg,), st_mm: (pv, sb), st_score: (sm,),
                        st_exp: (sm,), st_agg: (sm,), st_act4: (sm,),
                        st_fin: (sm,), st_tr: (ph,), st_copy: ()}
                for i in range(NB + 10):
                    for lag, fn in stages:
                        j = i - lag
                        if 0 <= j < NB:
                            fn(j, *args[fn])

        # ---------------------------- schedule ---------------------------
        with tc.tile_pool(name="xt", bufs=1) as xtp:
            xT_s = xtp.tile([D_IN, NLP], f32)
            nc.sync.dma_start(xT_s[:], xT_d[:])
            dense(1)

        nc.gpsimd.collective_compute(
            "AllGather", Op.bypass, replica_groups=[list(range(C))],
            ins=[bounce1[:]], outs=[table1[:]])

        edge(1)
        dense(2)

        nc.gpsimd.collective_compute(
            "AllGather", Op.bypass, replica_groups=[list(range(C))],
            ins=[bounce2[:]], outs=[table2[:]])

        edge(2)

    nc.compile()
    return nc


# ----------------------------------------------------------------------------
# entry point
# ----------------------------------------------------------------------------

def _make_in_maps(inputs):
    x = np.asarray(inputs["x"], np.float32)
    f1 = _fold(inputs["Wl1"], inputs["bl1"], inputs["Wr1"], inputs["br1"],
               inputs["We1"], inputs["att1"], inputs["bias1"])
    f2 = _fold(inputs["Wl2"], inputs["bl2"], inputs["Wr2"], inputs["br2"],
               inputs["We2"], inputs["att2"], inputs["bias2"],
               in_perm=f1["pi"], h_offset=True)
    g = _prep(x, inputs["edge_index"], inputs["edge_weight"])
    rhs1, rhs2, fake1, fake2 = _consts(g["KMAX"], f1["We"], f2["We"],
                                       f1["p1"], f2["p1"])

    plan = dict(g, p1=f1["p1"], p2=f2["p1"])
    Wl2b = np.concatenate([f2["WlT"], f2["blRow"]], 0).astype(np.float16)
    Wr2b = np.concatenate([f2["WrT"], f2["brRow"]], 0).astype(np.float16)
    shared = dict(
        Wl1T=f1["WlT"], Wr1T=f1["WrT"], bl1R=f1["blRow"], br1=f1["br"],
        Wl2b=Wl2b, Wr2b=Wr2b,
        rhs1=rhs1, rhs2=rhs2, fake1=fake1, fake2=fake2,
        inva1=f1["inva"][None, :], bias1=f1["bias"][None, :],
        inva2=f2["inva"][None, :], bias2=f2["bias"][None, :],
    )
    in_maps = []
    for c in range(C):
        m = dict(shared)
        m.update(xT=g["xT"][c], idxA=g["idxA"][c], idxB=g["idxB"][c],
                 wT=g["wT"][c])
        in_maps.append(m)
    return plan, in_maps, g, f2


def kernel(**inputs):
    from concourse.bass_utils import run_bass_kernel_spmd

    plan, in_maps, g, f2 = _make_in_maps(inputs)
    nc = _build(plan)
    res = run_bass_kernel_spmd(nc, in_maps, list(range(C)))

    full_new = np.concatenate([res.results[c]["out"][:NL] for c in range(C)], 0)
    full_old = full_new[g["new_id"]]
    out = np.empty((N, DO), np.float32)
    out[:, f2["pi"]] = full_old + 1e-4
    return out.astype(np.float32)


# revision 19
# speedup vs baseline: 1.2147x; 1.0035x over previous
"""Two-layer GATv2 (heads=1, edge_dim=1) on 8 Trainium2 NeuronCores.

Sharding: nodes dealt round-robin by in-degree onto 8 cores; dst-grouped
edges stay local; source features come from an AllGather'd full table via
dma_gather on three concurrent SWDGE queues (queues 1-3 dispatch in ~0.4us
and generate descriptors on their own Q7 core pairs in parallel).

Per 128-node (dst) block:
  v   = [xrT | w]-lhsT @ [IdTile | WeDiag] + ident @ g    -- 2 matmuls/chunk
  m   = prelu(v, 0.2)                                     -- ACT (from PSUM)
  s   = sum_{d<P1} m - sum_{d>=P1} m                      -- att sign-split
  e   = exp(s), Z = row-sum(e)                            -- ACT with accum
  agg = sum_k e_k * g_k                                   -- DVE mult+reduce
  out = agg * inva * (1/Z) + bias                         -- undo |att| scale
Padded gather slots point at a per-core "fake" table row holding -/+1e6 in
the sign-sorted layout, so exp(s)==0 exactly -- no mask tensors needed.
Dense phase computes xl node-major (lhsT = xT block) and xr feature-major
(lhsT = WrT) directly -- no PE transposes; biases fold into rank-1 matmul /
ACT bias. Layer-2 softplus runs on ACT; the +1e-4 is added host-side.
"""

import numpy as np

N, E, D_IN, DH, DO = 50000, 800000, 128, 64, 32
C = 8                      # cores
NL = N // C                # nodes per core (6250)
P = 128                    # partitions = nodes per block
NB = (NL + P - 1) // P     # blocks per core (49)
NLP = NB * P               # padded nodes per core (6272)
NLF = 6256                 # bounce rows per core (6250 real + fake + pad)
SPLIT = 32768              # int16 gather table split (table-row space)
NT = C * NLF               # table rows (50048)
VSMALL = 32                # K threshold for double-buffered PSUM v-tiles
FAKE_A = 6250              # core-0 fake row (side A)
FAKE_B = 5 * NLF + 6250 - SPLIT   # core-5 fake row offset in side B
GQ = (1, 2, 3)             # SWDGE queues (q0 ucode blocks the sequencer)


# ----------------------------------------------------------------------------
# host-side: weight folding and graph layout
# ----------------------------------------------------------------------------

def _fold(Wl, bl, Wr, br, We, att, bias, in_perm=None, h_offset=False):
    att = np.asarray(att, np.float64)
    pi = np.concatenate([np.nonzero(att >= 0)[0], np.nonzero(att < 0)[0]])
    p1 = int((att >= 0).sum())
    a = np.maximum(np.abs(att[pi]), 1e-30)
    Wl = np.asarray(Wl, np.float64)[pi] * a[:, None]
    Wr = np.asarray(Wr, np.float64)[pi] * a[:, None]
    bl = np.asarray(bl, np.float64)[pi] * a
    br = np.asarray(br, np.float64)[pi] * a
    We_ = np.asarray(We, np.float64)[pi, 0] * a
    if in_perm is not None:
        Wl = Wl[:, in_perm]
        Wr = Wr[:, in_perm]
    if h_offset:  # input arrives as h+1
        bl = bl - Wl.sum(1)
        br = br - Wr.sum(1)
    return dict(
        WlT=np.ascontiguousarray(Wl.T, np.float32),
        WrT=np.ascontiguousarray(Wr.T, np.float32),
        blRow=bl.astype(np.float32)[None, :],
        br=br.astype(np.float32)[:, None],
        brRow=br.astype(np.float32)[None, :],
        We=We_.astype(np.float32),
        inva=(1.0 / a).astype(np.float32),
        bias=np.asarray(bias, np.float64)[pi].astype(np.float32),
        pi=pi, p1=p1,
    )


def _prep(x, edge_index, edge_weight):
    src = np.asarray(edge_index[0], np.int64)
    dst = np.asarray(edge_index[1], np.int64)
    w = np.asarray(edge_weight, np.float32)

    deg = np.bincount(dst, minlength=N)
    wsum = np.bincount(dst, weights=w.astype(np.float64), minlength=N)
    loop_w = (wsum / np.maximum(deg, 1)).astype(np.float32)

    order = np.argsort(-deg, kind="stable")
    ranks = np.empty(N, np.int64)
    ranks[order] = np.arange(N)
    core = ranks % C
    crank = ranks // C
    new_id = core * NL + crank          # output-row space
    trow = core * NLF + crank           # gather-table-row space
    inv = np.empty(N, np.int64)
    inv[new_id] = np.arange(N)          # old id of each new id

    esrc = np.concatenate([trow[src], trow])          # table rows of sources
    edst = np.concatenate([new_id[dst], new_id])      # local ids of dsts
    ew = np.concatenate([w, loop_w]).astype(np.float32)

    side = (esrc >= SPLIT).astype(np.int64)
    eord = np.argsort(edst * 2 + side, kind="stable")
    sdst, ssrc, sw, sside = edst[eord], esrc[eord], ew[eord], side[eord]

    nA = np.bincount(edst[side == 0], minlength=N)
    nB = np.bincount(edst[side == 1], minlength=N)

    grp = np.searchsorted(sdst, np.arange(N))
    pos = np.arange(E + N) - grp[sdst]
    posAB = np.where(sside == 0, pos, pos - nA[sdst])

    nblk = (np.arange(N) % NL) // P      # block of each new id
    KA = np.zeros(NB, np.int64)
    KB = np.zeros(NB, np.int64)
    np.maximum.at(KA, nblk, nA)
    np.maximum.at(KB, nblk, nB)
    KA = np.maximum(KA, 1)
    K = KA + KB
    assert int(K.max()) * DH * 4 <= 16384, f"KMAX {K.max()} overflows PSUM"
    KMAX = int(K.max())
    totK = int(K.sum())

    e_core = sdst // NL
    e_loc = sdst % NL
    e_blk = e_loc // P
    e_p = e_loc % P
    e_k = np.where(sside == 0, posAB, KA[e_blk] + posAB)

    colsA = np.concatenate([[0], np.cumsum(KA * 8)]).astype(np.int64)
    colsB = np.concatenate([[0], np.cumsum(KB * 8)]).astype(np.int64)
    idxA = np.full((C, 128, int(colsA[-1])), FAKE_A, np.int16)
    idxB = np.full((C, 128, max(int(colsB[-1]), 16)), FAKE_B, np.int16)
    mA = sside == 0
    fA = e_k[mA] * P + e_p[mA]
    idxA[e_core[mA], fA % 16, colsA[e_blk[mA]] + fA // 16] = \
        ssrc[mA].astype(np.int16)
    mB = ~mA
    fB = (e_k[mB] - KA[e_blk[mB]]) * P + e_p[mB]
    idxB[e_core[mB], fB % 16, colsB[e_blk[mB]] + fB // 16] = \
        (ssrc[mB] - SPLIT).astype(np.int16)
    if NL % P:  # dummy partitions in last block: avoid Z=0 (point at row 0)
        for p in range(NL % P, P):
            f = 0 * P + p
            idxA[:, f % 16, colsA[NB - 1] + f // 16] = 0
    for rep in range(1, 8):
        idxA[:, 16 * rep:16 * rep + 16] = idxA[:, :16]
        idxB[:, 16 * rep:16 * rep + 16] = idxB[:, :16]

    x = np.asarray(x, np.float32)
    xT = np.zeros((C, D_IN, NLP), np.float32)
    perm = inv.reshape(C, NL)
    for c in range(C):
        xT[c, :, :NL] = x[perm[c]].T

    wT = np.zeros((C, KMAX, NLP), np.float16)
    wT[e_core, e_k, e_blk * P + e_p] = sw.astype(np.float16)

    return dict(new_id=new_id, K=K, KA=KA, KB=KB, KMAX=KMAX, totK=totK,
                colsA=colsA, colsB=colsB, idxA=idxA, idxB=idxB, xT=xT, wT=wT)


def _consts(KMAX, We1, We2, p1, p2):
    # layer 1: rhsCat1 = [IdTile1 (64 rows); WeDiag1 (KMAX rows)]
    H1 = DH + KMAX
    rhs1 = np.zeros((H1, KMAX * DH), np.float16)
    for d in range(DH):
        rhs1[d, d::DH] = 1.0
    for k in range(KMAX):
        rhs1[DH + k, k * DH:(k + 1) * DH] = We1
    # layer 2: rhsCat2 = [IdTile2 (32 rows); WeDiag2 (KMAX rows)], 64-col slots
    H2 = DO + KMAX
    rhs2 = np.zeros((H2, KMAX * DH), np.float16)
    for d in range(DO):
        rhs2[d, d::DH] = 1.0
    for k in range(KMAX):
        rhs2[DO + k, k * DH:k * DH + DO] = We2
    fake1 = np.where(np.arange(DH) < p1, -1e6, 1e6).astype(np.float32)[None, :]
    f2 = np.zeros(DH, np.float32)
    f2[:DO] = np.where(np.arange(DO) < p2, -1e6, 1e6)
    fake2 = f2[None, :]
    return rhs1, rhs2, fake1, fake2


# ----------------------------------------------------------------------------
# device program
# ----------------------------------------------------------------------------

def _build(plan):
    import concourse.bacc as bacc
    import concourse.bass as bass
    import concourse.mybir as mybir
    import concourse.tile as tile
    from concourse.library_config import mlp
    from concourse.masks import make_identity

    f32 = mybir.dt.float32
    f16 = mybir.dt.float16
    i16 = mybir.dt.int16
    Op = mybir.AluOpType
    Act = mybir.ActivationFunctionType

    K, KA, KB = plan["K"], plan["KA"], plan["KB"]
    KMAX, totK = plan["KMAX"], plan["totK"]
    colsA, colsB = plan["colsA"], plan["colsB"]
    P1, P2 = plan["p1"], plan["p2"]
    nA_cols = int(colsA[-1])
    nB_cols = max(int(colsB[-1]), 16)
    H1, H2 = DH + KMAX, DO + KMAX

    nc = bacc.Bacc("TRN2", debug=False, num_swdge_queues=4)

    def din(name, shape, dt=f32):
        return nc.dram_tensor(name, shape, dt, kind="ExternalInput")

    xT_d = din("xT", [D_IN, NLP])
    idxA_d = din("idxA", [128, nA_cols], i16)
    idxB_d = din("idxB", [128, nB_cols], i16)
    wT_d = din("wT", [KMAX, NLP], f16)
    rhs1_d = din("rhs1", [H1, KMAX * DH], f16)
    rhs2_d = din("rhs2", [H2, KMAX * DH], f16)
    Wl1T_d, Wr1T_d = din("Wl1T", [D_IN, DH]), din("Wr1T", [D_IN, DH])
    bl1R_d, br1_d = din("bl1R", [1, DH]), din("br1", [DH, 1])
    Wl2b_d = din("Wl2b", [DH + 1, DO], f16)   # [Wl2T; bl2 row]
    Wr2b_d = din("Wr2b", [DH + 1, DO], f16)   # [Wr2T; br2 row]
    inva1_d, bias1_d = din("inva1", [1, DH]), din("bias1", [1, DH])
    inva2_d, bias2_d = din("inva2", [1, DO]), din("bias2", [1, DO])
    fake1_d = din("fake1", [1, DH])
    fake2_d = din("fake2", [1, DH])

    out_d = nc.dram_tensor("out", [NLP, DO], f32, kind="ExternalOutput")

    bounce1 = nc.dram_tensor("bounce1", [NLF, DH], f32)
    table1 = nc.dram_tensor("table1", [NT, DH], f32)
    bounce2 = nc.dram_tensor("bounce2", [NLF, DH], f32)
    table2 = nc.dram_tensor("table2", [NT, DH], f32)

    with tile.TileContext(nc) as tc:
      with tc.tile_pool(name="persist", bufs=1) as pp:
        ident = pp.tile([P, P], f32)
        make_identity(nc, ident[:])
        nc.gpsimd.load_library(mlp)

        idxA_t = pp.tile([128, nA_cols], i16)
        idxB_t = pp.tile([128, nB_cols], i16)
        rhs1_t = pp.tile([H1, KMAX * DH], f16)
        rhs2_t = pp.tile([H2, KMAX * DH], f16)
        LT1 = pp.tile([H1, NLP], f16)     # rows 0:64 xr1T, 64:H1 wT
        LT2 = pp.tile([H2, NLP], f16)     # rows 0:32 xr2T, 32:H2 wT
        hT = pp.tile([DH + 1, NLP], f16)  # row 64 = ones (bias fold)
        t5all = pp.tile([P, NB * DO], f32)  # layer-2 pre-softplus outputs
        Wl1T_t = pp.tile([D_IN, DH], f32, tag="Wl1T_t")
        Wr1T_t = pp.tile([D_IN, DH], f32, tag="Wr1T_t")
        bl1R_t = pp.tile([1, DH], f32, tag="bl1R_t")
        br1_t = pp.tile([DH, 1], f32, tag="br1_t")
        Wl2b_t = pp.tile([DH + 1, DO], f16, tag="Wl2b_t")
        Wr2b_t = pp.tile([DH + 1, DO], f16, tag="Wr2b_t")
        ones1_t = pp.tile([1, P], f32, tag="ones1_t")
        inva1_t = pp.tile([P, DH], f32)
        bias1_t = pp.tile([P, DH], f32)
        inva2_t = pp.tile([P, DO], f32)
        bias2_t = pp.tile([P, DO], f32)
        fake1_t = pp.tile([1, DH], f32, tag="fake1_t")
        fake2_t = pp.tile([1, DH], f32, tag="fake2_t")

        nc.vector.memset(hT[DH:DH + 1, :], 1.0)
        nc.vector.memset(ones1_t[:], 1.0)

        for t, d in [(idxA_t, idxA_d), (idxB_t, idxB_d),
                     (rhs1_t, rhs1_d), (rhs2_t, rhs2_d),
                     (Wl1T_t, Wl1T_d), (Wr1T_t, Wr1T_d),
                     (bl1R_t, bl1R_d), (br1_t, br1_d),
                     (Wl2b_t, Wl2b_d), (Wr2b_t, Wr2b_d),
                     (fake1_t, fake1_d), (fake2_t, fake2_d)]:
            nc.sync.dma_start(t[:], d[:])
        nc.sync.dma_start(LT1[DH:DH + KMAX, :], wT_d[:])
        nc.sync.dma_start(LT2[DO:DO + KMAX, :], wT_d[:])
        for t, d, dd in [(inva1_t, inva1_d, DH), (bias1_t, bias1_d, DH),
                         (inva2_t, inva2_d, DO), (bias2_t, bias2_d, DO)]:
            nc.sync.dma_start(t[:], d[:].to_broadcast([P, dd]))
        nc.sync.dma_start(bounce1[6250:6251, :], fake1_t[:])
        nc.sync.dma_start(bounce2[6250:6251, :], fake2_t[:])

        def bcast_inner(ap, n):
            return bass.AP(ap.tensor, ap.offset, [*ap.ap, [0, n]])

        # ------------------ dense phase (either layer) -------------------
        def dense(layer):
            """xl node-major -> bounce rows; xr feature-major -> LT rows."""
            if layer == 1:
                DOUT, LT, bounce = DH, LT1, bounce1
            else:
                DOUT, LT, bounce = DO, LT2, bounce2
            with (
                tc.tile_pool(name=f"dps{layer}", bufs=2, space="PSUM") as dps,
                tc.tile_pool(name=f"dsb{layer}", bufs=3) as dsb,
            ):
                for j in range(NB):
                    c0 = j * P
                    if layer == 1:
                        lhs_blk = xT_s[:, c0:c0 + P]
                        ps = dps.tile([P, DOUT], f32, tag="xl", space="PSUM")
                        nc.tensor.matmul(out=ps[:], lhsT=lhs_blk,
                                         rhs=Wl1T_t[:], start=True, stop=False)
                        nc.tensor.matmul(out=ps[:], lhsT=ones1_t[:],
                                         rhs=bl1R_t[:], start=False, stop=True,
                                         skip_group_check=True)
                        ps2 = dps.tile([DOUT, P], f32, tag="xr", space="PSUM")
                        nc.tensor.matmul(out=ps2[:], lhsT=Wr1T_t[:],
                                         rhs=lhs_blk, start=True, stop=True)
                        nc.scalar.activation(out=LT[0:DOUT, c0:c0 + P],
                                             in_=ps2[:], func=Act.Identity,
                                             bias=br1_t[:])
                    else:
                        lhs_blk = hT[:, c0:c0 + P]
                        ps = dps.tile([P, DOUT], f32, tag="xl", space="PSUM")
                        nc.tensor.matmul(out=ps[:], lhsT=lhs_blk,
                                         rhs=Wl2b_t[:], start=True, stop=True)
                        ps2 = dps.tile([DOUT, P], f32, tag="xr", space="PSUM")
                        nc.tensor.matmul(out=ps2[:], lhsT=Wr2b_t[:],
                                         rhs=lhs_blk, start=True, stop=True)
                        nc.scalar.activation(out=LT[0:DOUT, c0:c0 + P],
                                             in_=ps2[:], func=Act.Identity,
                                             bias=0.0)
                    st = dsb.tile([P, DH], f32, tag="st")
                    if layer == 2:
                        nc.vector.memset(st[:], 0.0)
                    nc.scalar.activation(out=st[:, 0:DOUT], in_=ps[:],
                                         func=Act.Identity, bias=0.0)
                    lo, hi = j * P, min((j + 1) * P, NL)
                    if hi > lo:
                        nc.sync.dma_start(out=bounce[lo:hi, :],
                                          in_=st[:hi - lo, :])

        # ------------------ edge phase (either layer) --------------------
        qctr = [0]

        def edge(layer):
            if layer == 1:
                D, p1, table, LT, rhs_t, H = DH, P1, table1, LT1, rhs1_t, H1
                inva_rep, bias_rep = inva1_t, bias1_t
            else:
                D, p1, table, LT, rhs_t, H = DO, P2, table2, LT2, rhs2_t, H2
                inva_rep, bias_rep = inva2_t, bias2_t

            S = {}  # per-block tile state across pipeline stages

            def st_gather(j, sg):
                Kj, KAj, KBj = int(K[j]), int(KA[j]), int(KB[j])
                g_t = sg.tile([P, KMAX, DH], f32, tag="g")
                q = GQ[qctr[0] % len(GQ)]
                qctr[0] += 1
                nc.gpsimd.dma_gather(
                    g_t[:, 0:KAj, :], table[0:SPLIT, :],
                    idxA_t[:, int(colsA[j]):int(colsA[j + 1])],
                    P * KAj, P * KAj, DH, single_packet=False, queue_num=q)
                if KBj:
                    q = GQ[qctr[0] % len(GQ)]
                    qctr[0] += 1
                    nc.gpsimd.dma_gather(
                        g_t[:, KAj:Kj, :], table[SPLIT:NT, :],
                        idxB_t[:, int(colsB[j]):int(colsB[j + 1])],
                        P * KBj, P * KBj, DH, single_packet=False, queue_num=q)
                S[j] = dict(g=g_t)

            def st_mm(j, pv, sb):
                Kj = int(K[j])
                W = Kj * DH
                g_flat = S[j]["g"][:].rearrange("p k d -> p (k d)")
                lhs_blk = LT[:, j * P:(j + 1) * P]
                m_t = sb.tile([P, W], f16, tag="m")
                for g0 in range(0, W, 512):
                    g1 = min(g0 + 512, W)
                    ps_v = pv.tile([P, 512], f32, tag="v", space="PSUM")
                    nc.tensor.matmul(out=ps_v[:, 0:g1 - g0], lhsT=lhs_blk,
                                     rhs=rhs_t[:, g0:g1],
                                     start=True, stop=False)
                    nc.tensor.matmul(out=ps_v[:, 0:g1 - g0], lhsT=ident[:],
                                     rhs=g_flat[:, g0:g1],
                                     start=False, stop=True)
                    nc.scalar.activation(out=m_t[:, g0:g1],
                                         in_=ps_v[:, 0:g1 - g0],
                                         func=Act.Prelu, alpha=0.2)
                S[j]["m"] = m_t

            def st_score(j, sm):
                Kj = int(K[j])
                m3d = S[j]["m"][:].rearrange("p (k d) -> p k d", d=DH)
                sp_t = sm.tile([P, KMAX], f32, tag="sp")
                sn_t = sm.tile([P, KMAX], f32, tag="sn")
                s_t = sm.tile([P, KMAX], f32, tag="s")
                if 0 < p1:
                    nc.vector.tensor_reduce(out=sp_t[:, 0:Kj],
                                            in_=m3d[:, :, 0:p1],
                                            axis=mybir.AxisListType.X,
                                            op=Op.add)
                if p1 < D:
                    nc.vector.tensor_reduce(out=sn_t[:, 0:Kj],
                                            in_=m3d[:, :, p1:D],
                                            axis=mybir.AxisListType.X,
                                            op=Op.add)
                if 0 < p1 < D:
                    nc.vector.scalar_tensor_tensor(
                        out=s_t[:, 0:Kj], in0=sn_t[:, 0:Kj], scalar=-1.0,
                        in1=sp_t[:, 0:Kj], op0=Op.mult, op1=Op.add)
                elif p1 == D:
                    s_t = sp_t
                else:
                    nc.vector.tensor_scalar(out=s_t[:, 0:Kj],
                                            in0=sn_t[:, 0:Kj],
                                            scalar1=-1.0, scalar2=None,
                                            op0=Op.mult)
                S[j]["s"] = s_t

            def st_exp(j, sm):
                Kj = int(K[j])
                e_t = sm.tile([P, KMAX], f32, tag="e")
                Z_t = sm.tile([P, 1], f32, tag="Z")
                nc.scalar.activation(out=e_t[:, 0:Kj], in_=S[j]["s"][:, 0:Kj],
                                     func=Act.Exp, accum_out=Z_t[:])
                S[j]["e"] = e_t
                S[j]["Z"] = Z_t

            def st_agg(j, sm):
                Kj = int(K[j])
                W = Kj * DH
                g_flat = S[j]["g"][:].rearrange("p k d -> p (k d)")
                m_t = S[j]["m"]
                iZ_t = sm.tile([P, 1], f32, tag="iZ")
                nc.vector.reciprocal(out=iZ_t[:], in_=S[j]["Z"][:])
                # m is dead after sp/sn: reuse its space for e*g
                nc.vector.tensor_tensor(out=m_t[:], in0=g_flat[:, 0:W],
                                        in1=bcast_inner(S[j]["e"][:, 0:Kj],
                                                        DH),
                                        op=Op.mult)
                agg_t = sm.tile([P, DH], f32, tag="agg")
                nc.vector.tensor_reduce(
                    out=agg_t[:],
                    in_=m_t[:].rearrange("p (k d) -> p d k", d=DH),
                    axis=mybir.AxisListType.X, op=Op.add)
                t4_t = sm.tile([P, D], f32, tag="t4")
                nc.vector.tensor_tensor(out=t4_t[:], in0=agg_t[:, 0:D],
                                        in1=inva_rep[:], op=Op.mult)
                if layer == 1:
                    t5_t = sm.tile([P, D], f32, tag="t5")
                else:
                    t5_t = t5all[:, j * DO:(j + 1) * DO]
                nc.vector.scalar_tensor_tensor(
                    out=t5_t[:], in0=t4_t[:], scalar=iZ_t[:],
                    in1=bias_rep[:], op0=Op.mult, op1=Op.add)
                S[j]["t5"] = t5_t

            def st_act4(j, sm):
                if layer != 1:
                    return
                t5_t = S[j]["t5"]
                # elu(x)+1 = min(exp(x), 1) + max(x, 0)
                E_t = sm.tile([P, D], f32, tag="E")
                nc.scalar.activation(out=E_t[:], in_=t5_t[:], func=Act.Exp)
                r_t = sm.tile([P, D], f32, tag="r")
                nc.scalar.activation(out=r_t[:], in_=t5_t[:], func=Act.Relu)
                S[j]["E"], S[j]["r"] = E_t, r_t

            def st_fin(j, sm):
                if layer != 1:
                    return
                h_t = sm.tile([P, D], f32, tag="h")
                nc.vector.scalar_tensor_tensor(
                    out=h_t[:], in0=S[j]["E"][:], scalar=1.0,
                    in1=S[j]["r"][:], op0=Op.min, op1=Op.add)
                S[j]["h"] = h_t

            def st_tr(j, ph):
                if layer == 1:
                    pst = ph.tile([DH, P], f32, tag="ht", space="PSUM")
                    nc.tensor.transpose(out=pst[:], in_=S[j]["h"][:],
                                        identity=ident[:])
                    S[j]["pst"] = pst

            def st_copy(j):
                if layer == 1:
                    nc.scalar.activation(out=hT[0:DH, j * P:(j + 1) * P],
                                         in_=S[j]["pst"][:],
                                         func=Act.Identity, bias=0.0)
                S.pop(j)

            def softplus_sweep(spool):
                # softplus(z) = relu(z) + ln(1 + exp(-|z|)), batched over all
                # blocks so Exp/Ln act-table swaps happen once, not per block
                SW = NB * DO
                ab1 = spool.tile([P, SW], f32, tag="ab1")
                ab2 = spool.tile([P, SW], f32, tag="ab2")
                nc.scalar.activation(out=ab1[:], in_=t5all[:], func=Act.Abs)
                nc.scalar.activation(out=ab2[:], in_=ab1[:], func=Act.Exp,
                                     scale=-1.0)
                nc.scalar.activation(out=ab1[:], in_=ab2[:], func=Act.Ln,
                                     bias=1.0)
                nc.vector.scalar_tensor_tensor(
                    out=ab2[:], in0=t5all[:], scalar=0.0, in1=ab1[:],
                    op0=Op.max, op1=Op.add)
                for j in range(NB):
                    nc.sync.dma_start(
                        out=out_d[j * P:(j + 1) * P, :],
                        in_=ab2[:, j * DO:(j + 1) * DO])

            # lags relative to the loop index (stage runs on block i - lag)
            stages = [
                (4, st_exp),     # ACT, before prelus in ACT stream
                (7, st_act4),    # ACT
                (10, st_copy),   # ACT                (layer 1)
                (5, st_agg),     # DVE, before scores in DVE stream
                (8, st_fin),     # DVE (+ out DMA on layer 2)
                (9, st_tr),      # PE, before mms     (layer 1)
                (0, st_gather),  # GpSimd prefetch
                (3, st_mm),      # PE + ACT prelus
                (3, st_score),   # DVE
            ]
            with (
                tc.tile_pool(name=f"pv{layer}", bufs=5, space="PSUM") as pv,
                tc.tile_pool(name=f"ph{layer}", bufs=3, space="PSUM") as ph,
                tc.tile_pool(name=f"sg{layer}", bufs=7) as sg,
                tc.tile_pool(name=f"sb{layer}", bufs=5) as sb,
                tc.tile_pool(name=f"sm{layer}", bufs=5) as sm,
            ):
                args = {st_gather: (sg,), st_mm: (pv, sb), st_score: (sm,),
                        st_exp: (sm,), st_agg: (sm,), st_act4: (sm,),
                        st_fin: (sm,), st_tr: (ph,), st_copy: ()}
                for i in range(NB + 10):
                    for lag, fn in stages:
                        j = i - lag
                        if 0 <= j < NB:
                            fn(j, *args[fn])
            if layer == 2:
                with tc.tile_pool(name="spw", bufs=1) as spool:
                    softplus_sweep(spool)

        # ---------------------------- schedule ---------------------------
        with tc.tile_pool(name="xt", bufs=1) as xtp:
            xT_s = xtp.tile([D_IN, NLP], f32)
            nc.sync.dma_start(xT_s[:], xT_d[:])
            dense(1)

        nc.gpsimd.collective_compute(
            "AllGather", Op.bypass, replica_groups=[list(range(C))],
            ins=[bounce1[:]], outs=[table1[:]])

        edge(1)
        dense(2)

        nc.gpsimd.collective_compute(
            "AllGather", Op.bypass, replica_groups=[list(range(C))],
            ins=[bounce2[:]], outs=[table2[:]])

        edge(2)

    nc.compile()
    return nc


# ----------------------------------------------------------------------------
# entry point
# ----------------------------------------------------------------------------

def _make_in_maps(inputs):
    x = np.asarray(inputs["x"], np.float32)
    f1 = _fold(inputs["Wl1"], inputs["bl1"], inputs["Wr1"], inputs["br1"],
               inputs["We1"], inputs["att1"], inputs["bias1"])
    f2 = _fold(inputs["Wl2"], inputs["bl2"], inputs["Wr2"], inputs["br2"],
               inputs["We2"], inputs["att2"], inputs["bias2"],
               in_perm=f1["pi"], h_offset=True)
    g = _prep(x, inputs["edge_index"], inputs["edge_weight"])
    rhs1, rhs2, fake1, fake2 = _consts(g["KMAX"], f1["We"], f2["We"],
                                       f1["p1"], f2["p1"])

    plan = dict(g, p1=f1["p1"], p2=f2["p1"])
    Wl2b = np.concatenate([f2["WlT"], f2["blRow"]], 0).astype(np.float16)
    Wr2b = np.concatenate([f2["WrT"], f2["brRow"]], 0).astype(np.float16)
    shared = dict(
        Wl1T=f1["WlT"], Wr1T=f1["WrT"], bl1R=f1["blRow"], br1=f1["br"],
        Wl2b=Wl2b, Wr2b=Wr2b,
        rhs1=rhs1, rhs2=rhs2, fake1=fake1, fake2=fake2,
        inva1=f1["inva"][None, :], bias1=f1["bias"][None, :],
        inva2=f2["inva"][None, :], bias2=f2["bias"][None, :],
    )
    in_maps = []
    for c in range(C):
        m = dict(shared)
        m.update(xT=g["xT"][c], idxA=g["idxA"][c], idxB=g["idxB"][c],
                 wT=g["wT"][c])
        in_maps.append(m)
    return plan, in_maps, g, f2


def kernel(**inputs):
    from concourse.bass_utils import run_bass_kernel_spmd

    plan, in_maps, g, f2 = _make_in_maps(inputs)
    nc = _build(plan)
    res = run_bass_kernel_spmd(nc, in_maps, list(range(C)))

    full_new = np.concatenate([res.results[c]["out"][:NL] for c in range(C)], 0)
    full_old = full_new[g["new_id"]]
    out = np.empty((N, DO), np.float32)
    out[:, f2["pi"]] = full_old + 1e-4
    return out.astype(np.float32)


# revision 20
# speedup vs baseline: 1.2722x; 1.0473x over previous
"""Two-layer GATv2 (heads=1, edge_dim=1) on 8 Trainium2 NeuronCores.

Sharding: nodes dealt round-robin by in-degree onto 8 cores; dst-grouped
edges stay local; source features come from an AllGather'd full table via
dma_gather on three concurrent SWDGE queues (queues 1-3 dispatch in ~0.4us
and generate descriptors on their own Q7 core pairs in parallel).

Per 128-node (dst) block:
  v   = [xrT | w]-lhsT @ [IdTile | WeDiag] + ident @ g    -- 2 matmuls/chunk
  m   = prelu(v, 0.2)                                     -- ACT (from PSUM)
  s   = sum_{d<P1} m - sum_{d>=P1} m                      -- att sign-split
  e   = exp(s), Z = row-sum(e)                            -- ACT with accum
  agg = sum_k e_k * g_k                                   -- DVE mult+reduce
  out = agg * inva * (1/Z) + bias                         -- undo |att| scale
Padded gather slots point at a per-core "fake" table row holding -/+1e6 in
the sign-sorted layout, so exp(s)==0 exactly -- no mask tensors needed.
Dense phase computes xl node-major (lhsT = xT block) and xr feature-major
(lhsT = WrT) directly -- no PE transposes; biases fold into rank-1 matmul /
ACT bias. Layer-2 softplus runs on ACT; the +1e-4 is added host-side.
"""

import numpy as np

N, E, D_IN, DH, DO = 50000, 800000, 128, 64, 32
C = 8                      # cores
NL = N // C                # nodes per core (6250)
P = 128                    # partitions = nodes per block
NB = (NL + P - 1) // P     # blocks per core (49)
NLP = NB * P               # padded nodes per core (6272)
NLF = 6256                 # bounce rows per core (6250 real + fake + pad)
SPLIT = 32768              # int16 gather table split (table-row space)
NT = C * NLF               # table rows (50048)
VSMALL = 32                # K threshold for double-buffered PSUM v-tiles
FAKE_A = 6250              # core-0 fake row (side A)
FAKE_B = 5 * NLF + 6250 - SPLIT   # core-5 fake row offset in side B
GQ = (1, 2, 3)             # SWDGE queues (q0 ucode blocks the sequencer)


# ----------------------------------------------------------------------------
# host-side: weight folding and graph layout
# ----------------------------------------------------------------------------

def _fold(Wl, bl, Wr, br, We, att, bias, in_perm=None, h_offset=False):
    att = np.asarray(att, np.float64)
    pi = np.concatenate([np.nonzero(att >= 0)[0], np.nonzero(att < 0)[0]])
    p1 = int((att >= 0).sum())
    a = np.maximum(np.abs(att[pi]), 1e-30)
    Wl = np.asarray(Wl, np.float64)[pi] * a[:, None]
    Wr = np.asarray(Wr, np.float64)[pi] * a[:, None]
    bl = np.asarray(bl, np.float64)[pi] * a
    br = np.asarray(br, np.float64)[pi] * a
    We_ = np.asarray(We, np.float64)[pi, 0] * a
    if in_perm is not None:
        Wl = Wl[:, in_perm]
        Wr = Wr[:, in_perm]
    if h_offset:  # input arrives as h+1
        bl = bl - Wl.sum(1)
        br = br - Wr.sum(1)
    return dict(
        WlT=np.ascontiguousarray(Wl.T, np.float32),
        WrT=np.ascontiguousarray(Wr.T, np.float32),
        blRow=bl.astype(np.float32)[None, :],
        br=br.astype(np.float32)[:, None],
        brRow=br.astype(np.float32)[None, :],
        We=We_.astype(np.float32),
        inva=(1.0 / a).astype(np.float32),
        bias=np.asarray(bias, np.float64)[pi].astype(np.float32),
        pi=pi, p1=p1,
    )


def _prep(x, edge_index, edge_weight):
    src = np.asarray(edge_index[0], np.int64)
    dst = np.asarray(edge_index[1], np.int64)
    w = np.asarray(edge_weight, np.float32)

    deg = np.bincount(dst, minlength=N)
    wsum = np.bincount(dst, weights=w.astype(np.float64), minlength=N)
    loop_w = (wsum / np.maximum(deg, 1)).astype(np.float32)

    order = np.argsort(-deg, kind="stable")
    ranks = np.empty(N, np.int64)
    ranks[order] = np.arange(N)
    core = ranks % C
    crank = ranks // C
    new_id = core * NL + crank          # output-row space
    trow = core * NLF + crank           # gather-table-row space
    inv = np.empty(N, np.int64)
    inv[new_id] = np.arange(N)          # old id of each new id

    esrc = np.concatenate([trow[src], trow])          # table rows of sources
    edst = np.concatenate([new_id[dst], new_id])      # local ids of dsts
    ew = np.concatenate([w, loop_w]).astype(np.float32)

    side = (esrc >= SPLIT).astype(np.int64)
    eord = np.argsort(edst * 2 + side, kind="stable")
    sdst, ssrc, sw, sside = edst[eord], esrc[eord], ew[eord], side[eord]

    nA = np.bincount(edst[side == 0], minlength=N)
    nB = np.bincount(edst[side == 1], minlength=N)

    grp = np.searchsorted(sdst, np.arange(N))
    pos = np.arange(E + N) - grp[sdst]
    posAB = np.where(sside == 0, pos, pos - nA[sdst])

    nblk = (np.arange(N) % NL) // P      # block of each new id
    KA = np.zeros(NB, np.int64)
    KB = np.zeros(NB, np.int64)
    np.maximum.at(KA, nblk, nA)
    np.maximum.at(KB, nblk, nB)
    KA = np.maximum(KA, 1)
    K = KA + KB
    assert int(K.max()) * DH * 4 <= 16384, f"KMAX {K.max()} overflows PSUM"
    KMAX = int(K.max())
    totK = int(K.sum())

    e_core = sdst // NL
    e_loc = sdst % NL
    e_blk = e_loc // P
    e_p = e_loc % P
    e_k = np.where(sside == 0, posAB, KA[e_blk] + posAB)

    colsA = np.concatenate([[0], np.cumsum(KA * 8)]).astype(np.int64)
    colsB = np.concatenate([[0], np.cumsum(KB * 8)]).astype(np.int64)
    idxA = np.full((C, 128, int(colsA[-1])), FAKE_A, np.int16)
    idxB = np.full((C, 128, max(int(colsB[-1]), 16)), FAKE_B, np.int16)
    mA = sside == 0
    fA = e_k[mA] * P + e_p[mA]
    idxA[e_core[mA], fA % 16, colsA[e_blk[mA]] + fA // 16] = \
        ssrc[mA].astype(np.int16)
    mB = ~mA
    fB = (e_k[mB] - KA[e_blk[mB]]) * P + e_p[mB]
    idxB[e_core[mB], fB % 16, colsB[e_blk[mB]] + fB // 16] = \
        (ssrc[mB] - SPLIT).astype(np.int16)
    if NL % P:  # dummy partitions in last block: avoid Z=0 (point at row 0)
        for p in range(NL % P, P):
            f = 0 * P + p
            idxA[:, f % 16, colsA[NB - 1] + f // 16] = 0
    for rep in range(1, 8):
        idxA[:, 16 * rep:16 * rep + 16] = idxA[:, :16]
        idxB[:, 16 * rep:16 * rep + 16] = idxB[:, :16]

    x = np.asarray(x, np.float32)
    xT = np.zeros((C, D_IN, NLP), np.float32)
    perm = inv.reshape(C, NL)
    for c in range(C):
        xT[c, :, :NL] = x[perm[c]].T

    wT = np.zeros((C, KMAX, NLP), np.float16)
    wT[e_core, e_k, e_blk * P + e_p] = sw.astype(np.float16)

    return dict(new_id=new_id, K=K, KA=KA, KB=KB, KMAX=KMAX, totK=totK,
                colsA=colsA, colsB=colsB, idxA=idxA, idxB=idxB, xT=xT, wT=wT)


def _consts(KMAX, We1, We2, p1, p2):
    # layer 1: rhsCat1 = [IdTile1 (64 rows); WeDiag1 (KMAX rows)]
    H1 = DH + KMAX
    rhs1 = np.zeros((H1, KMAX * DH), np.float16)
    for d in range(DH):
        rhs1[d, d::DH] = 1.0
    for k in range(KMAX):
        rhs1[DH + k, k * DH:(k + 1) * DH] = We1
    # layer 2: rhsCat2 = [IdTile2 (32 rows); WeDiag2 (KMAX rows)], 64-col slots
    H2 = DO + KMAX
    rhs2 = np.zeros((H2, KMAX * DH), np.float16)
    for d in range(DO):
        rhs2[d, d::DH] = 1.0
    for k in range(KMAX):
        rhs2[DO + k, k * DH:k * DH + DO] = We2
    fake1 = np.where(np.arange(DH) < p1, -1e6, 1e6).astype(np.float32)[None, :]
    f2 = np.zeros(DH, np.float32)
    f2[:DO] = np.where(np.arange(DO) < p2, -1e6, 1e6)
    fake2 = f2[None, :]
    return rhs1, rhs2, fake1, fake2


# ----------------------------------------------------------------------------
# device program
# ----------------------------------------------------------------------------

def _build(plan):
    import concourse.bacc as bacc
    import concourse.bass as bass
    import concourse.mybir as mybir
    import concourse.tile as tile
    from concourse.library_config import mlp
    from concourse.masks import make_identity

    f32 = mybir.dt.float32
    f16 = mybir.dt.float16
    i16 = mybir.dt.int16
    Op = mybir.AluOpType
    Act = mybir.ActivationFunctionType

    K, KA, KB = plan["K"], plan["KA"], plan["KB"]
    KMAX, totK = plan["KMAX"], plan["totK"]
    colsA, colsB = plan["colsA"], plan["colsB"]
    P1, P2 = plan["p1"], plan["p2"]
    nA_cols = int(colsA[-1])
    nB_cols = max(int(colsB[-1]), 16)
    H1, H2 = DH + KMAX, DO + KMAX

    nc = bacc.Bacc("TRN2", debug=False, num_swdge_queues=4)

    def din(name, shape, dt=f32):
        return nc.dram_tensor(name, shape, dt, kind="ExternalInput")

    xT_d = din("xT", [D_IN, NLP])
    idxA_d = din("idxA", [128, nA_cols], i16)
    idxB_d = din("idxB", [128, nB_cols], i16)
    wT_d = din("wT", [KMAX, NLP], f16)
    rhs1_d = din("rhs1", [H1, KMAX * DH], f16)
    rhs2_d = din("rhs2", [H2, KMAX * DH], f16)
    Wl1T_d, Wr1T_d = din("Wl1T", [D_IN, DH]), din("Wr1T", [D_IN, DH])
    bl1R_d, br1_d = din("bl1R", [1, DH]), din("br1", [DH, 1])
    Wl2b_d = din("Wl2b", [DH + 1, DO], f16)   # [Wl2T; bl2 row]
    Wr2b_d = din("Wr2b", [DH + 1, DO], f16)   # [Wr2T; br2 row]
    inva1_d, bias1_d = din("inva1", [1, DH]), din("bias1", [1, DH])
    inva2_d, bias2_d = din("inva2", [1, DO]), din("bias2", [1, DO])
    fake1_d = din("fake1", [1, DH])
    fake2_d = din("fake2", [1, DH])

    out_d = nc.dram_tensor("out", [NLP, DO], f32, kind="ExternalOutput")

    bounce1 = nc.dram_tensor("bounce1", [NLF, DH], f32)
    table1 = nc.dram_tensor("table1", [NT, DH], f32)
    bounce2 = nc.dram_tensor("bounce2", [NLF, DH], f32)
    table2 = nc.dram_tensor("table2", [NT, DH], f32)

    with tile.TileContext(nc) as tc:
      with tc.tile_pool(name="persist", bufs=1) as pp:
        ident = pp.tile([P, P], f32)
        make_identity(nc, ident[:])
        nc.gpsimd.load_library(mlp)

        idxA_t = pp.tile([128, nA_cols], i16)
        idxB_t = pp.tile([128, nB_cols], i16)
        hT = pp.tile([DH + 1, NLP], f16)  # row 64 = ones (bias fold)
        Wl1T_t = pp.tile([D_IN, DH], f32, tag="Wl1T_t")
        Wr1T_t = pp.tile([D_IN, DH], f32, tag="Wr1T_t")
        bl1R_t = pp.tile([1, DH], f32, tag="bl1R_t")
        br1_t = pp.tile([DH, 1], f32, tag="br1_t")
        Wl2b_t = pp.tile([DH + 1, DO], f16, tag="Wl2b_t")
        Wr2b_t = pp.tile([DH + 1, DO], f16, tag="Wr2b_t")
        ones1_t = pp.tile([1, P], f32, tag="ones1_t")
        inva1_t = pp.tile([P, DH], f32)
        bias1_t = pp.tile([P, DH], f32)
        inva2_t = pp.tile([P, DO], f32)
        bias2_t = pp.tile([P, DO], f32)
        fake1_t = pp.tile([1, DH], f32, tag="fake1_t")
        fake2_t = pp.tile([1, DH], f32, tag="fake2_t")

        nc.vector.memset(hT[DH:DH + 1, :], 1.0)
        nc.vector.memset(ones1_t[:], 1.0)

        for t, d in [(idxA_t, idxA_d), (idxB_t, idxB_d),
                     (Wl1T_t, Wl1T_d), (Wr1T_t, Wr1T_d),
                     (bl1R_t, bl1R_d), (br1_t, br1_d),
                     (Wl2b_t, Wl2b_d), (Wr2b_t, Wr2b_d),
                     (fake1_t, fake1_d), (fake2_t, fake2_d)]:
            nc.sync.dma_start(t[:], d[:])
        for t, d, dd in [(inva1_t, inva1_d, DH), (bias1_t, bias1_d, DH),
                         (inva2_t, inva2_d, DO), (bias2_t, bias2_d, DO)]:
            nc.sync.dma_start(t[:], d[:].to_broadcast([P, dd]))
        nc.sync.dma_start(bounce1[6250:6251, :], fake1_t[:])
        nc.sync.dma_start(bounce2[6250:6251, :], fake2_t[:])

        def bcast_inner(ap, n):
            return bass.AP(ap.tensor, ap.offset, [*ap.ap, [0, n]])

        # ------------------ dense phase (either layer) -------------------
        def dense(layer, LT):
            """xl node-major -> bounce rows; xr feature-major -> LT rows."""
            if layer == 1:
                DOUT, bounce = DH, bounce1
            else:
                DOUT, bounce = DO, bounce2
            with (
                tc.tile_pool(name=f"dps{layer}", bufs=2, space="PSUM") as dps,
                tc.tile_pool(name=f"dsb{layer}", bufs=3) as dsb,
            ):
                for j in range(NB):
                    c0 = j * P
                    if layer == 1:
                        lhs_blk = xT_s[:, c0:c0 + P]
                        ps = dps.tile([P, DOUT], f32, tag="xl", space="PSUM")
                        nc.tensor.matmul(out=ps[:], lhsT=lhs_blk,
                                         rhs=Wl1T_t[:], start=True, stop=False)
                        nc.tensor.matmul(out=ps[:], lhsT=ones1_t[:],
                                         rhs=bl1R_t[:], start=False, stop=True,
                                         skip_group_check=True)
                        ps2 = dps.tile([DOUT, P], f32, tag="xr", space="PSUM")
                        nc.tensor.matmul(out=ps2[:], lhsT=Wr1T_t[:],
                                         rhs=lhs_blk, start=True, stop=True)
                        nc.scalar.activation(out=LT[0:DOUT, c0:c0 + P],
                                             in_=ps2[:], func=Act.Identity,
                                             bias=br1_t[:])
                    else:
                        lhs_blk = hT[:, c0:c0 + P]
                        ps = dps.tile([P, DOUT], f32, tag="xl", space="PSUM")
                        nc.tensor.matmul(out=ps[:], lhsT=lhs_blk,
                                         rhs=Wl2b_t[:], start=True, stop=True)
                        ps2 = dps.tile([DOUT, P], f32, tag="xr", space="PSUM")
                        nc.tensor.matmul(out=ps2[:], lhsT=Wr2b_t[:],
                                         rhs=lhs_blk, start=True, stop=True)
                        nc.scalar.activation(out=LT[0:DOUT, c0:c0 + P],
                                             in_=ps2[:], func=Act.Identity,
                                             bias=0.0)
                    st = dsb.tile([P, DH], f32, tag="st")
                    if layer == 2:
                        nc.vector.memset(st[:], 0.0)
                    nc.scalar.activation(out=st[:, 0:DOUT], in_=ps[:],
                                         func=Act.Identity, bias=0.0)
                    lo, hi = j * P, min((j + 1) * P, NL)
                    if hi > lo:
                        nc.sync.dma_start(out=bounce[lo:hi, :],
                                          in_=st[:hi - lo, :])

        # ------------------ edge phase (either layer) --------------------
        qctr = [0]

        def edge(layer, LT, rhs_t, t5all=None):
            if layer == 1:
                D, p1, table, H = DH, P1, table1, H1
                inva_rep, bias_rep = inva1_t, bias1_t
            else:
                D, p1, table, H = DO, P2, table2, H2
                inva_rep, bias_rep = inva2_t, bias2_t

            S = {}  # per-block tile state across pipeline stages

            def st_gather(j, sg):
                Kj, KAj, KBj = int(K[j]), int(KA[j]), int(KB[j])
                g_t = sg.tile([P, KMAX, DH], f32, tag="g")
                q = GQ[qctr[0] % len(GQ)]
                qctr[0] += 1
                nc.gpsimd.dma_gather(
                    g_t[:, 0:KAj, :], table[0:SPLIT, :],
                    idxA_t[:, int(colsA[j]):int(colsA[j + 1])],
                    P * KAj, P * KAj, DH, single_packet=False, queue_num=q)
                if KBj:
                    q = GQ[qctr[0] % len(GQ)]
                    qctr[0] += 1
                    nc.gpsimd.dma_gather(
                        g_t[:, KAj:Kj, :], table[SPLIT:NT, :],
                        idxB_t[:, int(colsB[j]):int(colsB[j + 1])],
                        P * KBj, P * KBj, DH, single_packet=False, queue_num=q)
                S[j] = dict(g=g_t)

            def st_mm(j, pv, sb):
                Kj = int(K[j])
                W = Kj * DH
                g_flat = S[j]["g"][:].rearrange("p k d -> p (k d)")
                lhs_blk = LT[:, j * P:(j + 1) * P]
                m_t = sb.tile([P, W], f16, tag="m")
                for g0 in range(0, W, 512):
                    g1 = min(g0 + 512, W)
                    ps_v = pv.tile([P, 512], f32, tag="v", space="PSUM")
                    nc.tensor.matmul(out=ps_v[:, 0:g1 - g0], lhsT=lhs_blk,
                                     rhs=rhs_t[:, g0:g1],
                                     start=True, stop=False)
                    nc.tensor.matmul(out=ps_v[:, 0:g1 - g0], lhsT=ident[:],
                                     rhs=g_flat[:, g0:g1],
                                     start=False, stop=True)
                    nc.scalar.activation(out=m_t[:, g0:g1],
                                         in_=ps_v[:, 0:g1 - g0],
                                         func=Act.Prelu, alpha=0.2)
                S[j]["m"] = m_t

            def st_score(j, sm):
                Kj = int(K[j])
                m3d = S[j]["m"][:].rearrange("p (k d) -> p k d", d=DH)
                sp_t = sm.tile([P, KMAX], f32, tag="sp")
                sn_t = sm.tile([P, KMAX], f32, tag="sn")
                s_t = sm.tile([P, KMAX], f32, tag="s")
                if 0 < p1:
                    nc.vector.tensor_reduce(out=sp_t[:, 0:Kj],
                                            in_=m3d[:, :, 0:p1],
                                            axis=mybir.AxisListType.X,
                                            op=Op.add)
                if p1 < D:
                    nc.vector.tensor_reduce(out=sn_t[:, 0:Kj],
                                            in_=m3d[:, :, p1:D],
                                            axis=mybir.AxisListType.X,
                                            op=Op.add)
                if 0 < p1 < D:
                    nc.vector.scalar_tensor_tensor(
                        out=s_t[:, 0:Kj], in0=sn_t[:, 0:Kj], scalar=-1.0,
                        in1=sp_t[:, 0:Kj], op0=Op.mult, op1=Op.add)
                elif p1 == D:
                    s_t = sp_t
                else:
                    nc.vector.tensor_scalar(out=s_t[:, 0:Kj],
                                            in0=sn_t[:, 0:Kj],
                                            scalar1=-1.0, scalar2=None,
                                            op0=Op.mult)
                S[j]["s"] = s_t

            def st_exp(j, sm):
                Kj = int(K[j])
                e_t = sm.tile([P, KMAX], f32, tag="e")
                Z_t = sm.tile([P, 1], f32, tag="Z")
                nc.scalar.activation(out=e_t[:, 0:Kj], in_=S[j]["s"][:, 0:Kj],
                                     func=Act.Exp, accum_out=Z_t[:])
                S[j]["e"] = e_t
                S[j]["Z"] = Z_t

            def st_agg(j, sm):
                Kj = int(K[j])
                W = Kj * DH
                g_flat = S[j]["g"][:].rearrange("p k d -> p (k d)")
                m_t = S[j]["m"]
                iZ_t = sm.tile([P, 1], f32, tag="iZ")
                nc.vector.reciprocal(out=iZ_t[:], in_=S[j]["Z"][:])
                # m is dead after sp/sn: reuse its space for e*g
                nc.vector.tensor_tensor(out=m_t[:], in0=g_flat[:, 0:W],
                                        in1=bcast_inner(S[j]["e"][:, 0:Kj],
                                                        DH),
                                        op=Op.mult)
                agg_t = sm.tile([P, DH], f32, tag="agg")
                nc.vector.tensor_reduce(
                    out=agg_t[:],
                    in_=m_t[:].rearrange("p (k d) -> p d k", d=DH),
                    axis=mybir.AxisListType.X, op=Op.add)
                t4_t = sm.tile([P, D], f32, tag="t4")
                nc.vector.tensor_tensor(out=t4_t[:], in0=agg_t[:, 0:D],
                                        in1=inva_rep[:], op=Op.mult)
                if layer == 1:
                    t5_t = sm.tile([P, D], f32, tag="t5")
                else:
                    t5_t = t5all[:, j * DO:(j + 1) * DO]
                nc.vector.scalar_tensor_tensor(
                    out=t5_t[:], in0=t4_t[:], scalar=iZ_t[:],
                    in1=bias_rep[:], op0=Op.mult, op1=Op.add)
                S[j]["t5"] = t5_t

            def st_act4(j, sm):
                if layer != 1:
                    return
                t5_t = S[j]["t5"]
                # elu(x)+1 = min(exp(x), 1) + max(x, 0)
                E_t = sm.tile([P, D], f32, tag="E")
                nc.scalar.activation(out=E_t[:], in_=t5_t[:], func=Act.Exp)
                r_t = sm.tile([P, D], f32, tag="r")
                nc.scalar.activation(out=r_t[:], in_=t5_t[:], func=Act.Relu)
                S[j]["E"], S[j]["r"] = E_t, r_t

            def st_fin(j, sm):
                if layer != 1:
                    return
                h_t = sm.tile([P, D], f32, tag="h")
                nc.vector.scalar_tensor_tensor(
                    out=h_t[:], in0=S[j]["E"][:], scalar=1.0,
                    in1=S[j]["r"][:], op0=Op.min, op1=Op.add)
                S[j]["h"] = h_t

            def st_tr(j, ph):
                if layer == 1:
                    pst = ph.tile([DH, P], f32, tag="ht", space="PSUM")
                    nc.tensor.transpose(out=pst[:], in_=S[j]["h"][:],
                                        identity=ident[:])
                    S[j]["pst"] = pst

            def st_copy(j):
                if layer == 1:
                    nc.scalar.activation(out=hT[0:DH, j * P:(j + 1) * P],
                                         in_=S[j]["pst"][:],
                                         func=Act.Identity, bias=0.0)
                S.pop(j)

            def softplus_sweep(spool):
                # softplus(z) = relu(z) + ln(1 + exp(-|z|)), batched over all
                # blocks so Exp/Ln act-table swaps happen once, not per block
                SW = NB * DO
                ab1 = spool.tile([P, SW], f32, tag="ab1")
                ab2 = spool.tile([P, SW], f32, tag="ab2")
                nc.scalar.activation(out=ab1[:], in_=t5all[:], func=Act.Abs)
                nc.scalar.activation(out=ab2[:], in_=ab1[:], func=Act.Exp,
                                     scale=-1.0)
                nc.scalar.activation(out=ab1[:], in_=ab2[:], func=Act.Ln,
                                     bias=1.0)
                nc.vector.scalar_tensor_tensor(
                    out=ab2[:], in0=t5all[:], scalar=0.0, in1=ab1[:],
                    op0=Op.max, op1=Op.add)
                for j in range(NB):
                    nc.sync.dma_start(
                        out=out_d[j * P:(j + 1) * P, :],
                        in_=ab2[:, j * DO:(j + 1) * DO])

            # lags relative to the loop index (stage runs on block i - lag)
            stages = [
                (5, st_exp),     # ACT, before prelus in ACT stream
                (8, st_act4),    # ACT
                (11, st_copy),   # ACT                (layer 1)
                (6, st_agg),     # DVE, before scores in DVE stream
                (9, st_fin),     # DVE
                (10, st_tr),     # PE, before mms     (layer 1)
                (0, st_gather),  # GpSimd prefetch
                (4, st_mm),      # PE + ACT prelus
                (4, st_score),   # DVE
            ]
            with (
                tc.tile_pool(name=f"pv{layer}", bufs=5, space="PSUM") as pv,
                tc.tile_pool(name=f"ph{layer}", bufs=3, space="PSUM") as ph,
                tc.tile_pool(name=f"sg{layer}", bufs=10) as sg,
                tc.tile_pool(name=f"sb{layer}", bufs=4) as sb,
                tc.tile_pool(name=f"sm{layer}", bufs=5) as sm,
            ):
                args = {st_gather: (sg,), st_mm: (pv, sb), st_score: (sm,),
                        st_exp: (sm,), st_agg: (sm,), st_act4: (sm,),
                        st_fin: (sm,), st_tr: (ph,), st_copy: ()}
                for i in range(NB + 11):
                    for lag, fn in stages:
                        j = i - lag
                        if 0 <= j < NB:
                            fn(j, *args[fn])
            if layer == 2:
                with tc.tile_pool(name="spw", bufs=1) as spool:
                    softplus_sweep(spool)

        # ---------------------------- schedule ---------------------------
        with tc.tile_pool(name="l1p", bufs=1) as l1p:
            LT1 = l1p.tile([H1, NLP], f16)     # rows 0:64 xr1T, 64:H1 wT
            rhs1_t = l1p.tile([H1, KMAX * DH], f16)
            nc.sync.dma_start(LT1[DH:DH + KMAX, :], wT_d[:])
            nc.sync.dma_start(rhs1_t[:], rhs1_d[:])
            with tc.tile_pool(name="xt", bufs=1) as xtp:
                xT_s = xtp.tile([D_IN, NLP], f32)
                nc.sync.dma_start(xT_s[:], xT_d[:])
                dense(1, LT1)

            nc.gpsimd.collective_compute(
                "AllGather", Op.bypass, replica_groups=[list(range(C))],
                ins=[bounce1[:]], outs=[table1[:]])

            edge(1, LT1, rhs1_t)

        with tc.tile_pool(name="l2p", bufs=1) as l2p:
            LT2 = l2p.tile([H2, NLP], f16)     # rows 0:32 xr2T, 32:H2 wT
            rhs2_t = l2p.tile([H2, KMAX * DH], f16)
            t5all = l2p.tile([P, NB * DO], f32)
            nc.sync.dma_start(LT2[DO:DO + KMAX, :], wT_d[:])
            nc.sync.dma_start(rhs2_t[:], rhs2_d[:])
            dense(2, LT2)

            nc.gpsimd.collective_compute(
                "AllGather", Op.bypass, replica_groups=[list(range(C))],
                ins=[bounce2[:]], outs=[table2[:]])

            edge(2, LT2, rhs2_t, t5all)

    nc.compile()
    return nc


# ----------------------------------------------------------------------------
# entry point
# ----------------------------------------------------------------------------

def _make_in_maps(inputs):
    x = np.asarray(inputs["x"], np.float32)
    f1 = _fold(inputs["Wl1"], inputs["bl1"], inputs["Wr1"], inputs["br1"],
               inputs["We1"], inputs["att1"], inputs["bias1"])
    f2 = _fold(inputs["Wl2"], inputs["bl2"], inputs["Wr2"], inputs["br2"],
               inputs["We2"], inputs["att2"], inputs["bias2"],
               in_perm=f1["pi"], h_offset=True)
    g = _prep(x, inputs["edge_index"], inputs["edge_weight"])
    rhs1, rhs2, fake1, fake2 = _consts(g["KMAX"], f1["We"], f2["We"],
                                       f1["p1"], f2["p1"])

    plan = dict(g, p1=f1["p1"], p2=f2["p1"])
    Wl2b = np.concatenate([f2["WlT"], f2["blRow"]], 0).astype(np.float16)
    Wr2b = np.concatenate([f2["WrT"], f2["brRow"]], 0).astype(np.float16)
    shared = dict(
        Wl1T=f1["WlT"], Wr1T=f1["WrT"], bl1R=f1["blRow"], br1=f1["br"],
        Wl2b=Wl2b, Wr2b=Wr2b,
        rhs1=rhs1, rhs2=rhs2, fake1=fake1, fake2=fake2,
        inva1=f1["inva"][None, :], bias1=f1["bias"][None, :],
        inva2=f2["inva"][None, :], bias2=f2["bias"][None, :],
    )
    in_maps = []
    for c in range(C):
        m = dict(shared)
        m.update(xT=g["xT"][c], idxA=g["idxA"][c], idxB=g["idxB"][c],
                 wT=g["wT"][c])
        in_maps.append(m)
    return plan, in_maps, g, f2


def kernel(**inputs):
    from concourse.bass_utils import run_bass_kernel_spmd

    plan, in_maps, g, f2 = _make_in_maps(inputs)
    nc = _build(plan)
    res = run_bass_kernel_spmd(nc, in_maps, list(range(C)))

    full_new = np.concatenate([res.results[c]["out"][:NL] for c in range(C)], 0)
    full_old = full_new[g["new_id"]]
    out = np.empty((N, DO), np.float32)
    out[:, f2["pi"]] = full_old + 1e-4
    return out.astype(np.float32)
